# revision 1
# baseline (speedup 1.0000x reference)
"""Causal self-attention (B=4, T=2048, C=1024, H=16) on 8 trn2 NeuronCores.

Sharding: core = (batch b, head-half s).  Each core computes q/k/v
projections for its 8 heads (weights pre-sliced/transposed on host),
causal flash-style attention with transposed score tiles, and a partial
(row-sharded) c_proj.  Host gather sums the two partials per batch.

Device data layout (all fp32):
  xT    [1024, 2048]  x[b].T                      (in-ch on partitions)
  wqkT  [1024, 1024]  [Wq_local | Wk_local].T     (in-ch on partitions)
  bqk   [128, 8]      q/k bias, per out-ch block
  wvT   [1024, 512]   Wv_local.T
  wpT   [512, 1024]   Wproj[:, local].T
  bpj   [128, 8]      bproj + bv@WprojT (folded), half of it per core
  zT    [1024, 2048]  partial output, transposed
"""

import os
import sys

sys.path.insert(0, "/opt/trn_rl_repo")

import numpy as np

B, T, C, H = 4, 2048, 1024, 16
D = 64          # head dim
NH = 8          # heads per core
LC = NH * D     # local channels = 512
P = 128
QT = 512        # query tile (also matmul moving free dim)
NQT = T // QT   # 4
NKB = T // P    # 16 key blocks
IC = C // P     # 8 input-channel blocks

# matmul input dtype: float32r = full-rate PE mode (reduced precision),
# float32 = exact but 4x slower.
MM_DT = os.environ.get("BASS_ATTN_MM_DT", "float32r")

_nc_cache = {}


def _build_nc():
    from contextlib import ExitStack

    import concourse.bass as bass  # noqa: F401
    import concourse.mybir as mybir
    from concourse import bacc, tile

    f32 = mybir.dt.float32
    mdt = getattr(mybir.dt, MM_DT)
    Exp = mybir.ActivationFunctionType.Exp
    Copy = mybir.ActivationFunctionType.Copy
    is_ge = mybir.AluOpType.is_ge

    def c(ap):
        return ap

    nc = bacc.Bacc("TRN2", target_bir_lowering=False, debug=False, num_devices=8)
    xT = nc.dram_tensor("xT", [C, T], mdt, kind="ExternalInput").ap()
    wqkT = nc.dram_tensor("wqkT", [C, 2 * LC], mdt, kind="ExternalInput").ap()
    bqk = nc.dram_tensor("bqk", [P, 2 * LC // P], f32, kind="ExternalInput").ap()
    wvT = nc.dram_tensor("wvT", [C, LC], mdt, kind="ExternalInput").ap()
    wpT = nc.dram_tensor("wpT", [LC, C], mdt, kind="ExternalInput").ap()
    bpj = nc.dram_tensor("bpj", [P, C // P], f32, kind="ExternalInput").ap()
    zT = nc.dram_tensor("zT", [C, T], f32, kind="ExternalOutput").ap()

    with tile.TileContext(nc) as tc:
        with ExitStack() as outer:
            persist = outer.enter_context(tc.tile_pool(name="persist", bufs=1))
            # qk_sb: out-ch blocks 0-3 = q, 4-7 = k; [out-ch 128, tok 2048]
            qk_sb = [persist.tile([P, T], mdt, tag=f"qk{i}", name=f"qk{i}") for i in range(8)]
            # v_sb[kb]: [tok 128, head 8, d 64 + ones col]
            v_sb = [persist.tile([P, NH, D + 1], mdt, tag=f"v{i}", name=f"v{i}") for i in range(NKB)]
            bqk_sb = persist.tile([P, 8], f32, tag="bqk")
            bpj_sb = persist.tile([P, 8], f32, tag="bpj")
            nc.sync.dma_start(bqk_sb[:], bqk)
            nc.sync.dma_start(bpj_sb[:], bpj)

            # ---- Stage A/B: qk projection + v projection, streaming x ----
            with tc.tile_pool(name="wts", bufs=1) as wpool, \
                 tc.tile_pool(name="xs", bufs=2) as xpool, \
                 tc.tile_pool(name="psab", bufs=4, space="PSUM") as pspool:
                wqk_sb = [wpool.tile([P, 2 * LC], mdt, tag=f"wqk{i}", name=f"wqk{i}") for i in range(IC)]
                wv_sb = [wpool.tile([P, LC], mdt, tag=f"wv{i}", name=f"wv{i}") for i in range(IC)]
                for i in range(IC):
                    nc.sync.dma_start(wqk_sb[i][:], wqkT[i * P:(i + 1) * P, :])
                    nc.sync.dma_start(wv_sb[i][:], wvT[i * P:(i + 1) * P, :])
                for tt in range(NQT):
                    xt = [xpool.tile([P, QT], mdt, tag=f"x{i}", name=f"x{i}") for i in range(IC)]
                    for i in range(IC):
                        nc.sync.dma_start(
                            xt[i][:], xT[i * P:(i + 1) * P, tt * QT:(tt + 1) * QT])
                    # qk-proj: psum[out-ch 128, tok 512] accumulated over in-ch
                    for oc in range(8):
                        ps = pspool.tile([P, QT], f32, tag="psA")
                        for i in range(IC):
                            nc.tensor.matmul(
                                ps[:], c(wqk_sb[i][:, oc * P:(oc + 1) * P]),
                                c(xt[i][:]), start=(i == 0), stop=(i == IC - 1))
                        nc.vector.tensor_scalar_add(
                            qk_sb[oc][:, tt * QT:(tt + 1) * QT], ps[:],
                            bqk_sb[:, oc:oc + 1])
                    # v-proj: psum[tok 128, out-ch 512] per tok block
                    for tb in range(4):
                        kb = tt * 4 + tb
                        ps = pspool.tile([P, NH, D], f32, tag="psB")
                        for i in range(IC):
                            nc.tensor.matmul(
                                ps[:], c(xt[i][:, tb * P:(tb + 1) * P]),
                                c(wv_sb[i][:]), start=(i == 0), stop=(i == IC - 1))
                        nc.scalar.activation(v_sb[kb][:, :, 0:D], ps[:], Copy)
                        # ones column for the softmax-denominator row of att@V
                        nc.scalar.activation(
                            v_sb[kb][:, :, D:D + 1], ps[:, :, 0:1],
                            mybir.ActivationFunctionType.Identity,
                            bias=1.0, scale=0.0)

            # ---- Stage C: attention;  Stage D: c_proj ----
            with tc.tile_pool(name="wp", bufs=1) as wppool, \
                 tc.tile_pool(name="ybuf", bufs=1) as ypool, \
                 tc.tile_pool(name="att", bufs=4) as apool, \
                 tc.tile_pool(name="pss", bufs=2, space="PSUM") as ps_s_pool, \
                 tc.tile_pool(name="pso", bufs=2, space="PSUM") as ps_o_pool, \
                 tc.tile_pool(name="nrm", bufs=4) as nrm_pool, \
                 tc.tile_pool(name="yraw", bufs=8) as yrawpool, \
                 tc.tile_pool(name="ptb", bufs=1, space="PSUM") as ps_b_pool, \
                 tc.tile_pool(name="psz", bufs=1, space="PSUM") as ps_z_pool, \
                 tc.tile_pool(name="zev", bufs=3) as zpool:
                wp_sb = [wppool.tile([P, C], mdt, tag=f"wp{i}", name=f"wp{i}") for i in range(4)]
                # y_sb: attention out, [local-ch 128, tok 2048] x 4 blocks
                y_sb = [ypool.tile([P, T], mdt, tag=f"y{i}", name=f"y{i}") for i in range(4)]
                for i in range(4):
                    nc.sync.dma_start(wp_sb[i][:], wpT[i * P:(i + 1) * P, :])
                # triangular mask (keep j >= p), shared by all diagonal blocks
                maskf = wppool.tile([P, QT], f32, tag="maskf", name="maskf")
                nc.vector.memset(maskf[:], 1.0)
                nc.gpsimd.affine_select(
                    maskf[:], maskf[:], compare_op=is_ge, fill=0.0,
                    base=0, pattern=[[1, QT]], channel_multiplier=-1)
                # all-ones column block, lhsT of the R-broadcast matmuls
                ones_sb = wppool.tile([P, D], f32, tag="ones", name="ones")
                nc.scalar.activation(
                    ones_sb[:], wp_sb[0][:, 0:D],
                    mybir.ActivationFunctionType.Identity, bias=1.0, scale=0.0)
                for qtt in range(NQT):
                    # S rows live at 32-aligned partitions (engine AP rule)
                    sgs = [nrm_pool.tile([P, QT], f32, tag=f"sg{i}", bufs=1,
                                         name=f"sg{i}") for i in range(2)]
                    rgs = [nrm_pool.tile([P, QT], f32, tag=f"rg{i}", bufs=1,
                                         name=f"rg{i}") for i in range(2)]
                    for i in range(2):
                        nc.vector.memset(sgs[i][:], 1.0)
                    yraws = []
                    for h in range(NH):
                        p0 = (h % 2) * D
                        qt_i = h // 2
                        kt_i = 4 + h // 2
                        nkb = (qtt + 1) * 4
                        po = ps_o_pool.tile([D + 1, QT], f32, tag="po")
                        # process kb in pairs sharing one PSUM tile + one exp
                        for pi in range(nkb // 2):
                            kbs = (2 * pi, 2 * pi + 1)
                            ns, c0s = [], []
                            for kb in kbs:
                                e = kb * P - qtt * QT
                                c0s.append(max(e, 0))
                                ns.append(QT - max(e, 0))
                            # pack both live column ranges into one tile; each
                            # matmul's output must stay inside one 512-col bank
                            o0 = 0
                            o1 = ns[0] if ns[0] + ns[1] <= QT else QT
                            width = o1 + ns[1]
                            ps = ps_s_pool.tile([P, 2 * QT], f32, tag="ps")
                            at = apool.tile([P, 2 * QT], mdt, tag="at")
                            for kb, n, c0, o in zip(kbs, ns, c0s, (o0, o1)):
                                nc.tensor.matmul(
                                    ps[:, o:o + n],
                                    c(qk_sb[kt_i][p0:p0 + D,
                                                  kb * P:(kb + 1) * P]),
                                    c(qk_sb[qt_i][p0:p0 + D,
                                                  qtt * QT + c0:(qtt + 1) * QT]),
                                    start=True, stop=True)
                            nc.scalar.activation(at[:, 0:width], ps[:, 0:width],
                                                 Exp, scale=0.125)
                            for kb, n, c0, o in zip(kbs, ns, c0s, (o0, o1)):
                                if kb * P - qtt * QT >= 0:
                                    # zero strict upper triangle; it never
                                    # reaches past the first 128 live columns
                                    m = min(n, P)
                                    nc.vector.tensor_mul(at[:, o:o + m],
                                                         at[:, o:o + m],
                                                         maskf[:, 0:m])
                                nc.tensor.matmul(
                                    po[:, c0:QT], c(v_sb[kb][:, h, :]),
                                    c(at[:, o:o + n]),
                                    start=(kb == 0), stop=(kb == nkb - 1))
                        # evict numerator+sums to SBUF, release the PSUM bank
                        yraw = yrawpool.tile([D + 1, QT], f32, tag="yraw")
                        nc.vector.tensor_copy(yraw[:], po[:])
                        r0 = 32 * (h % 4)
                        nc.vector.tensor_copy(sgs[h // 4][r0:r0 + 1, :],
                                              yraw[D:D + 1, :])
                        yraws.append(yraw)
                    # two reciprocals cover all 8 heads of this query tile
                    for i in range(2):
                        nc.vector.reciprocal(rgs[i][:], sgs[i][:])
                    for h in range(NH):
                        p0 = (h % 2) * D
                        r0 = 32 * (h % 4)
                        r_ap = rgs[h // 4][r0:r0 + 1, :]
                        if r0 == 96:  # matmul operands must start at 0/32/64
                            rfix = nrm_pool.tile([1, QT], f32, tag="rfix", bufs=2)
                            nc.vector.tensor_copy(rfix[:], r_ap)
                            r_ap = rfix[:]
                            r0 = 0
                        # broadcast R across 64 partitions via a K=1 matmul
                        btp = ps_b_pool.tile([D, QT], f32, tag="btp")
                        nc.tensor.matmul(
                            btp[:], ones_sb[r0:r0 + 1, 0:D],
                            r_ap, start=True, stop=True)
                        nc.vector.tensor_mul(
                            y_sb[h // 2][p0:p0 + D, qtt * QT:(qtt + 1) * QT],
                            yraws[h][0:D, :], btp[:])
                    # c_proj for this token tile becomes ready as soon as all
                    # heads of qtt are done; gives the PE gap-filler work
                    tt = qtt
                    for oc in range(8):
                        ps = ps_z_pool.tile([P, QT], f32, tag="pz")
                        for i in range(4):
                            nc.tensor.matmul(
                                ps[:], c(wp_sb[i][:, oc * P:(oc + 1) * P]),
                                c(y_sb[i][:, tt * QT:(tt + 1) * QT]),
                                start=(i == 0), stop=(i == 3))
                        zt = zpool.tile([P, QT], f32, tag="zt")
                        nc.vector.tensor_scalar_add(zt[:], ps[:], bpj_sb[:, oc:oc + 1])
                        nc.sync.dma_start(
                            zT[oc * P:(oc + 1) * P, tt * QT:(tt + 1) * QT], zt[:])
    nc.compile()
    return nc


def get_nc():
    if "nc" not in _nc_cache:
        _nc_cache["nc"] = _build_nc()
    return _nc_cache["nc"]


def _mm_np_dtype():
    if MM_DT == "bfloat16":
        import ml_dtypes
        return np.dtype(ml_dtypes.bfloat16)
    return np.dtype(np.float32)


def make_in_maps(x, Wqkv, bqkv, Wproj, bproj):
    x = np.asarray(x, np.float32)
    Wqkv = np.asarray(Wqkv, np.float32)
    bqkv = np.asarray(bqkv, np.float32)
    Wproj = np.asarray(Wproj, np.float32)
    bproj = np.asarray(bproj, np.float32)
    Wq, Wk, Wv = Wqkv[0:C], Wqkv[C:2 * C], Wqkv[2 * C:3 * C]
    bq, bk, bv = bqkv[0:C], bqkv[C:2 * C], bqkv[2 * C:3 * C]
    mdt = _mm_np_dtype()
    in_maps = []
    for b in range(B):
        xTb = np.ascontiguousarray(x[b].T.astype(mdt))
        for s in range(2):
            cols = slice(s * LC, (s + 1) * LC)
            wqkT = np.ascontiguousarray(
                np.concatenate([Wq[cols], Wk[cols]], 0).T.astype(mdt))
            bqk_ = np.concatenate([bq[cols], bk[cols]])
            wvT_ = np.ascontiguousarray(Wv[cols].T.astype(mdt))
            wpT_ = np.ascontiguousarray(Wproj[:, cols].T.astype(mdt))
            bp_eff = bv[cols] @ Wproj[:, cols].T
            if s == 0:
                bp_eff = bp_eff + bproj
            in_maps.append({
                "xT": xTb,
                "wqkT": wqkT,
                "bqk": np.ascontiguousarray(bqk_.reshape(8, P).T),
                "wvT": wvT_,
                "wpT": wpT_,
                "bpj": np.ascontiguousarray(bp_eff.astype(np.float32).reshape(8, P).T),
            })
    return in_maps


def gather_out(results):
    out = np.empty((B, T, C), np.float32)
    for b in range(B):
        zt = results[2 * b]["zT"] + results[2 * b + 1]["zT"]
        out[b] = zt.T
    return out


def kernel(x, Wqkv, bqkv, Wproj, bproj):
    from concourse.bass_utils import run_bass_kernel_spmd

    in_maps = make_in_maps(x, Wqkv, bqkv, Wproj, bproj)
    try:
        res = run_bass_kernel_spmd(get_nc(), in_maps, core_ids=list(range(8)))
    except Exception:
        # transient device faults have been observed once; retry a single time
        res = run_bass_kernel_spmd(get_nc(), in_maps, core_ids=list(range(8)))
    return gather_out(res.results)



# revision 5
# speedup vs baseline: 1.3199x; 1.3199x over previous
"""Causal self-attention (B=4, T=2048, C=1024, H=16) on 8 trn2 NeuronCores.

Sharding: core = (batch b, head-half s).  Each core computes q/k/v
projections for its 8 heads (weights pre-sliced/transposed on host),
causal flash-style attention with transposed score tiles, and a partial
(row-sharded) c_proj.  Host gather sums the two partials per batch.

v2: bf16 matmul inputs, proj/attention merged per token tile so the
scheduler can gap-fill, attention software-pipelined (S-pair emitted one
slot ahead of its att@V consumer so the PE never waits on exp), mask
DMA'd from host, paired-head normalization broadcast.

Device data layout:
  xT    [1024, 2048] bf16  x[b].T                  (in-ch on partitions)
  wqkT  [1024, 1024] bf16  [Wq_local | Wk_local].T
  bqk   [128, 8]     f32   q/k bias, per out-ch block
  wvT   [1024, 512]  bf16  Wv_local.T
  wpT   [512, 1024]  bf16  Wproj[:, local].T
  bpj   [128, 8]     f32   bproj + bv@WprojT (folded), half per core
  mask  [128, 512]   bf16  causal keep-mask m[p,c] = (c >= p)
  zT    [1024, 2048] f32   partial output, transposed
"""

import os
import sys

sys.path.insert(0, "/opt/trn_rl_repo")

import numpy as np

B, T, C, H = 4, 2048, 1024, 16
D = 64          # head dim
NH = 8          # heads per core
LC = NH * D     # local channels = 512
P = 128
QT = 512        # query tile (also matmul moving free dim)
NQT = T // QT   # 4
NKB = T // P    # 16 key blocks
IC = C // P     # 8 input-channel blocks

# matmul input dtype: bfloat16 = full-rate, float32r fallback (exact-ish)
MM_DT = os.environ.get("BASS_ATTN_MM_DT", "bfloat16")

_nc_cache = {}


def _build_nc():
    from contextlib import ExitStack

    import concourse.bass as bass  # noqa: F401
    import concourse.mybir as mybir
    from concourse import bacc, tile

    f32 = mybir.dt.float32
    mdt = getattr(mybir.dt, MM_DT)
    Exp = mybir.ActivationFunctionType.Exp

    nc = bacc.Bacc("TRN2", target_bir_lowering=False, debug=False, num_devices=8)
    xT = nc.dram_tensor("xT", [C, T], mdt, kind="ExternalInput").ap()
    wqkT = nc.dram_tensor("wqkT", [C, 2 * LC], mdt, kind="ExternalInput").ap()
    bqk = nc.dram_tensor("bqk", [P, 2 * LC // P], f32, kind="ExternalInput").ap()
    wvT = nc.dram_tensor("wvT", [C, LC], mdt, kind="ExternalInput").ap()
    wpT = nc.dram_tensor("wpT", [LC, C], mdt, kind="ExternalInput").ap()
    bpj = nc.dram_tensor("bpj", [P, C // P], f32, kind="ExternalInput").ap()
    maskd = nc.dram_tensor("mask", [P, QT], mdt, kind="ExternalInput").ap()
    zT = nc.dram_tensor("zT", [C, T], f32, kind="ExternalOutput").ap()

    with tile.TileContext(nc) as tc:
        with ExitStack() as st:
            st.enter_context(nc.allow_low_precision(
                reason="bf16 throughput; accuracy checked vs reference"))
            persist = st.enter_context(tc.tile_pool(name="persist", bufs=1))
            # qk_sb: out-ch blocks 0-3 = q, 4-7 = k; [out-ch 128, tok 2048]
            qk_sb = [persist.tile([P, T], mdt, tag=f"qk{i}", name=f"qk{i}")
                     for i in range(8)]
            # v_sb[kb]: [tok 128, head 8, d 64 + ones col]
            v_sb = [persist.tile([P, NH, D + 1], mdt, tag=f"v{i}", name=f"v{i}")
                    for i in range(NKB)]
            # y_sb: attention out, [local-ch 128, tok 2048] x 4 blocks
            y_sb = [persist.tile([P, T], mdt, tag=f"y{i}", name=f"y{i}")
                    for i in range(4)]
            bqk_sb = persist.tile([P, 8], f32, tag="bqk", name="bqk")
            bpj_sb = persist.tile([P, 8], f32, tag="bpj", name="bpj")
            maskf = persist.tile([P, QT], mdt, tag="maskf", name="maskf")
            # softmax-sum rows: 4 heads per tile at 32-aligned partitions;
            # memset once so dead rows never produce inf/NaN via reciprocal
            sgs = [persist.tile([P, QT], f32, tag=f"sg{i}", name=f"sg{i}")
                   for i in range(2)]
            rgs = [persist.tile([P, QT], mdt, tag=f"rg{i}", name=f"rg{i}")
                   for i in range(2)]
            # E matrices: block row-broadcast for paired-head normalization.
            # btp[p,c] = sum_k E[k,p]*rg[k,c]; E_lo rows {0,32}, E_hi {64,96}
            E_bc = [persist.tile([P, P], mdt, tag=f"E{i}", name=f"E{i}")
                    for i in range(2)]
            # weights
            wqk_sb = [persist.tile([P, 2 * LC], mdt, tag=f"wqk{i}",
                                   name=f"wqk{i}") for i in range(IC)]
            wv_sb = [persist.tile([P, LC], mdt, tag=f"wv{i}", name=f"wv{i}")
                     for i in range(IC)]
            wp_sb = [persist.tile([P, C], mdt, tag=f"wp{i}", name=f"wp{i}")
                     for i in range(4)]

            xpool = st.enter_context(tc.tile_pool(name="xs", bufs=2))
            apool = st.enter_context(tc.tile_pool(name="att", bufs=3))
            yrpool = st.enter_context(tc.tile_pool(name="yraw", bufs=4))
            ztpool = st.enter_context(tc.tile_pool(name="zev", bufs=3))
            # PSUM: pab 2 + ps 2x2 + po 1 + misc 1 = 8 banks
            pab = st.enter_context(tc.tile_pool(name="pab", bufs=2, space="PSUM"))
            pss = st.enter_context(tc.tile_pool(name="pss", bufs=2, space="PSUM"))
            pso = st.enter_context(tc.tile_pool(name="pso", bufs=1, space="PSUM"))
            psm = st.enter_context(tc.tile_pool(name="psm", bufs=1, space="PSUM"))

            # ---- weight / const DMAs (interleaved for fast first matmul) ----
            for i in range(IC):
                nc.sync.dma_start(wqk_sb[i][:], wqkT[i * P:(i + 1) * P, :])
                nc.sync.dma_start(wv_sb[i][:], wvT[i * P:(i + 1) * P, :])
            for i in range(4):
                nc.sync.dma_start(wp_sb[i][:], wpT[i * P:(i + 1) * P, :])
            nc.sync.dma_start(bqk_sb[:], bqk)
            nc.sync.dma_start(bpj_sb[:], bpj)
            nc.sync.dma_start(maskf[:], maskd)
            for i in range(2):
                nc.vector.memset(sgs[i][:], 1.0)
                nc.vector.memset(E_bc[i][:], 0.0)
            nc.vector.memset(E_bc[0][0:1, 0:D], 1.0)
            nc.vector.memset(E_bc[0][32:33, D:P], 1.0)
            nc.vector.memset(E_bc[1][64:65, 0:D], 1.0)
            nc.vector.memset(E_bc[1][96:97, D:P], 1.0)

            for tt in range(NQT):
                # ---- projections for token tile tt ----
                xt = [xpool.tile([P, QT], mdt, tag=f"x{i}", name=f"x{i}")
                      for i in range(IC)]
                for i in range(IC):
                    nc.sync.dma_start(
                        xt[i][:], xT[i * P:(i + 1) * P, tt * QT:(tt + 1) * QT])
                # qk-proj: psum[out-ch 128, tok 512] accumulated over in-ch
                for oc in range(8):
                    ps = pab.tile([P, QT], f32, tag="pab", name="pab")
                    for i in range(IC):
                        nc.tensor.matmul(
                            ps[:], wqk_sb[i][:, oc * P:(oc + 1) * P],
                            xt[i][:], start=(i == 0), stop=(i == IC - 1))
                    nc.vector.tensor_scalar_add(
                        qk_sb[oc][:, tt * QT:(tt + 1) * QT], ps[:],
                        bqk_sb[:, oc:oc + 1])
                # v-proj: psum[tok 128, out-ch 512] per tok block
                for tb in range(4):
                    kb = tt * 4 + tb
                    ps = pab.tile([P, NH, D], f32, tag="pab", name="pab")
                    for i in range(IC):
                        nc.tensor.matmul(
                            ps[:], xt[i][:, tb * P:(tb + 1) * P],
                            wv_sb[i][:], start=(i == 0), stop=(i == IC - 1))
                    nc.vector.tensor_copy(v_sb[kb][:, :, 0:D], ps[:])
                    nc.vector.memset(v_sb[kb][:, :, D:D + 1], 1.0)

                # ---- attention for query tile tt ----
                qtt = tt
                nkb = (qtt + 1) * 4
                npair = nkb // 2
                # task list: (h, pi); S+exp emitted one slot ahead of att@V
                tasks = [(h, pi) for h in range(NH) for pi in range(npair)]
                po_t = [None] * NH
                at_t = {}
                yr2 = [None] * 4

                def emit_s(h, pi):
                    p0 = (h % 2) * D
                    qt_i = h // 2
                    kt_i = 4 + h // 2
                    kbs = (2 * pi, 2 * pi + 1)
                    ns, c0s = [], []
                    for kb in kbs:
                        e = kb * P - qtt * QT
                        c0s.append(max(e, 0))
                        ns.append(QT - max(e, 0))
                    o1 = ns[0] if ns[0] + ns[1] <= QT else QT
                    width = o1 + ns[1]
                    ps = pss.tile([P, 2 * QT], f32, tag="ps", name="ps")
                    at = apool.tile([P, 2 * QT], mdt, tag="at", name="at")
                    for kb, n, c0, o in zip(kbs, ns, c0s, (0, o1)):
                        nc.tensor.matmul(
                            ps[:, o:o + n],
                            qk_sb[kt_i][p0:p0 + D, kb * P:(kb + 1) * P],
                            qk_sb[qt_i][p0:p0 + D,
                                        qtt * QT + c0:(qtt + 1) * QT],
                            start=True, stop=True)
                    nc.scalar.activation(at[:, 0:width], ps[:, 0:width],
                                         Exp, scale=0.125)
                    for kb, n, c0, o in zip(kbs, ns, c0s, (0, o1)):
                        if kb * P - qtt * QT >= 0:
                            # zero strict upper triangle; never reaches past
                            # the first 128 live columns
                            m = min(n, P)
                            nc.vector.tensor_mul(at[:, o:o + m],
                                                 at[:, o:o + m],
                                                 maskf[:, 0:m])
                    at_t[(h, pi)] = (at, ns, c0s, (0, o1))

                def emit_a(h, pi):
                    at, ns, c0s, os_ = at_t.pop((h, pi))
                    kbs = (2 * pi, 2 * pi + 1)
                    if pi == 0:
                        po_t[h] = pso.tile([D + 1, QT], f32, tag="po", name="po")
                    po = po_t[h]
                    for kb, n, c0, o in zip(kbs, ns, c0s, os_):
                        nc.tensor.matmul(
                            po[:, c0:QT], v_sb[kb][:, h, :], at[:, o:o + n],
                            start=(kb == 0), stop=(kb == nkb - 1))
                    if pi == npair - 1:
                        # evict numerator into the paired-head tile and the
                        # denominator row into its sg slot; frees the bank
                        j = h // 2
                        if yr2[j] is None:
                            yr2[j] = yrpool.tile([P, QT], f32, tag="yr", name="yr")
                        r0 = D * (h % 2)
                        nc.vector.tensor_copy(yr2[j][r0:r0 + D, :], po[0:D, :])
                        nc.vector.tensor_copy(
                            sgs[h // 4][32 * (h % 4):32 * (h % 4) + 1, :],
                            po[D:D + 1, :])
                        po_t[h] = None

                for s, t in enumerate(tasks):
                    emit_s(*t)
                    if s >= 1:
                        emit_a(*tasks[s - 1])
                emit_a(*tasks[-1])

                for g in range(2):
                    nc.vector.reciprocal(rgs[g][:], sgs[g][:])
                for j in range(4):
                    # heads 2j,2j+1 live in sg tile j//2 rows 64(j%2)+{0,32}
                    btp = psm.tile([P, QT], f32, tag="psm", name="btp")
                    nc.tensor.matmul(btp[:], E_bc[j % 2][:], rgs[j // 2][:],
                                     start=True, stop=True)
                    nc.vector.tensor_mul(
                        y_sb[j][:, qtt * QT:(qtt + 1) * QT], yr2[j][:], btp[:])

                # ---- c_proj for token tile tt ----
                for oc in range(8):
                    ps = pab.tile([P, QT], f32, tag="pab", name="pab")
                    for i in range(4):
                        nc.tensor.matmul(
                            ps[:], wp_sb[i][:, oc * P:(oc + 1) * P],
                            y_sb[i][:, tt * QT:(tt + 1) * QT],
                            start=(i == 0), stop=(i == 3))
                    zt = ztpool.tile([P, QT], f32, tag="zt", name="zt")
                    nc.vector.tensor_scalar_add(zt[:], ps[:],
                                                bpj_sb[:, oc:oc + 1])
                    nc.sync.dma_start(
                        zT[oc * P:(oc + 1) * P, tt * QT:(tt + 1) * QT], zt[:])
    nc.compile()
    return nc


def get_nc():
    if "nc" not in _nc_cache:
        _nc_cache["nc"] = _build_nc()
    return _nc_cache["nc"]


def _mm_np_dtype():
    if MM_DT == "bfloat16":
        import ml_dtypes
        return np.dtype(ml_dtypes.bfloat16)
    return np.dtype(np.float32)


def make_in_maps(x, Wqkv, bqkv, Wproj, bproj):
    x = np.asarray(x, np.float32)
    Wqkv = np.asarray(Wqkv, np.float32)
    bqkv = np.asarray(bqkv, np.float32)
    Wproj = np.asarray(Wproj, np.float32)
    bproj = np.asarray(bproj, np.float32)
    Wq, Wk, Wv = Wqkv[0:C], Wqkv[C:2 * C], Wqkv[2 * C:3 * C]
    bq, bk, bv = bqkv[0:C], bqkv[C:2 * C], bqkv[2 * C:3 * C]
    mdt = _mm_np_dtype()
    mask = (np.arange(QT)[None, :] >= np.arange(P)[:, None]).astype(mdt)
    in_maps = []
    for b in range(B):
        xTb = np.ascontiguousarray(x[b].T.astype(mdt))
        for s in range(2):
            cols = slice(s * LC, (s + 1) * LC)
            wqkT = np.ascontiguousarray(
                np.concatenate([Wq[cols], Wk[cols]], 0).T.astype(mdt))
            bqk_ = np.concatenate([bq[cols], bk[cols]])
            wvT_ = np.ascontiguousarray(Wv[cols].T.astype(mdt))
            wpT_ = np.ascontiguousarray(Wproj[:, cols].T.astype(mdt))
            bp_eff = bv[cols] @ Wproj[:, cols].T
            if s == 0:
                bp_eff = bp_eff + bproj
            in_maps.append({
                "xT": xTb,
                "wqkT": wqkT,
                "bqk": np.ascontiguousarray(bqk_.reshape(8, P).T),
                "wvT": wvT_,
                "wpT": wpT_,
                "bpj": np.ascontiguousarray(
                    bp_eff.astype(np.float32).reshape(8, P).T),
                "mask": mask,
            })
    return in_maps


def gather_out(results):
    out = np.empty((B, T, C), np.float32)
    for b in range(B):
        zt = results[2 * b]["zT"] + results[2 * b + 1]["zT"]
        out[b] = zt.T
    return out


def kernel(x, Wqkv, bqkv, Wproj, bproj):
    from concourse.bass_utils import run_bass_kernel_spmd

    in_maps = make_in_maps(x, Wqkv, bqkv, Wproj, bproj)
    try:
        res = run_bass_kernel_spmd(get_nc(), in_maps, core_ids=list(range(8)))
    except Exception:
        # transient device faults have been observed once; retry a single time
        res = run_bass_kernel_spmd(get_nc(), in_maps, core_ids=list(range(8)))
    return gather_out(res.results)


# revision 8
# speedup vs baseline: 1.4517x; 1.0999x over previous
"""Causal self-attention (B=4, T=2048, C=1024, H=16) on 8 trn2 NeuronCores.

Sharding: core = (batch b, head-half s).  Each core computes q/k/v
projections for its 8 heads (weights pre-sliced/transposed on host),
causal flash-style attention with transposed score tiles, and a partial
(row-sharded) c_proj.  Host gather sums the two partials per batch.

v2: bf16 matmul inputs, proj/attention merged per token tile so the
scheduler can gap-fill, attention software-pipelined (S-pair emitted one
slot ahead of its att@V consumer so the PE never waits on exp), mask
DMA'd from host, paired-head normalization broadcast.

Device data layout:
  xT    [1024, 2048] bf16  x[b].T                  (in-ch on partitions)
  wqkT  [1024, 1024] bf16  [Wq_local | Wk_local].T
  bqk   [128, 8]     f32   q/k bias, per out-ch block
  wvT   [1024, 512]  bf16  Wv_local.T
  wpT   [512, 1024]  bf16  Wproj[:, local].T
  bpj   [128, 8]     f32   bproj + bv@WprojT (folded), half per core
  mask  [128, 512]   bf16  causal keep-mask m[p,c] = (c >= p)
  zT    [1024, 2048] f32   partial output, transposed
"""

import os
import sys

sys.path.insert(0, "/opt/trn_rl_repo")

import numpy as np

B, T, C, H = 4, 2048, 1024, 16
D = 64          # head dim
NH = 8          # heads per core
LC = NH * D     # local channels = 512
P = 128
QT = 512        # query tile (also matmul moving free dim)
NQT = T // QT   # 4
NKB = T // P    # 16 key blocks
IC = C // P     # 8 input-channel blocks

# matmul input dtype: bfloat16 = full-rate, float32r fallback (exact-ish)
MM_DT = os.environ.get("BASS_ATTN_MM_DT", "bfloat16")

_nc_cache = {}


def _build_nc():
    from contextlib import ExitStack

    import concourse.bass as bass  # noqa: F401
    import concourse.mybir as mybir
    from concourse import bacc, tile

    f32 = mybir.dt.float32
    mdt = getattr(mybir.dt, MM_DT)
    Exp = mybir.ActivationFunctionType.Exp
    Copy = mybir.ActivationFunctionType.Copy

    nc = bacc.Bacc("TRN2", target_bir_lowering=False, debug=False, num_devices=8)
    xT = nc.dram_tensor("xT", [C, T], mdt, kind="ExternalInput").ap()
    wqkT = nc.dram_tensor("wqkT", [C, 2 * LC], mdt, kind="ExternalInput").ap()
    bqk = nc.dram_tensor("bqk", [P, 2 * LC // P], f32, kind="ExternalInput").ap()
    wvT = nc.dram_tensor("wvT", [C, LC], mdt, kind="ExternalInput").ap()
    wpT = nc.dram_tensor("wpT", [LC, C], mdt, kind="ExternalInput").ap()
    bpj = nc.dram_tensor("bpj", [P, C // P], f32, kind="ExternalInput").ap()
    maskd = nc.dram_tensor("mask", [P, QT], mdt, kind="ExternalInput").ap()
    zT = nc.dram_tensor("zT", [C, T], f32, kind="ExternalOutput").ap()

    with tile.TileContext(nc) as tc:
        with ExitStack() as st:
            st.enter_context(nc.allow_low_precision(
                reason="bf16 throughput; accuracy checked vs reference"))
            persist = st.enter_context(tc.tile_pool(name="persist", bufs=1))
            # qk_sb: out-ch blocks 0-3 = q, 4-7 = k; [out-ch 128, tok 2048]
            qk_sb = [persist.tile([P, T], mdt, tag=f"qk{i}", name=f"qk{i}")
                     for i in range(8)]
            # v_sb[kb]: [tok 128, head 8, d 64 + ones col]
            v_sb = [persist.tile([P, NH, D + 1], mdt, tag=f"v{i}", name=f"v{i}")
                    for i in range(NKB)]
            # y_sb: attention out, [local-ch 128, tok 2048] x 4 blocks
            y_sb = [persist.tile([P, T], mdt, tag=f"y{i}", name=f"y{i}")
                    for i in range(4)]
            bqk_sb = persist.tile([P, 8], f32, tag="bqk", name="bqk")
            bpj_sb = persist.tile([P, 8], f32, tag="bpj", name="bpj")
            maskf = persist.tile([P, QT], mdt, tag="maskf", name="maskf")
            # softmax-sum rows: 4 heads per tile at 32-aligned partitions;
            # memset once so dead rows never produce inf/NaN via reciprocal
            sgs = [persist.tile([P, QT], f32, tag=f"sg{i}", name=f"sg{i}")
                   for i in range(2)]
            rgs = [persist.tile([P, QT], mdt, tag=f"rg{i}", name=f"rg{i}")
                   for i in range(2)]
            # E matrices: block row-broadcast for paired-head normalization.
            # btp[p,c] = sum_k E[k,p]*rg[k,c]; E_lo rows {0,32}, E_hi {64,96}
            E_bc = [persist.tile([P, P], mdt, tag=f"E{i}", name=f"E{i}")
                    for i in range(2)]
            # weights
            wqk_sb = [persist.tile([P, 2 * LC], mdt, tag=f"wqk{i}",
                                   name=f"wqk{i}") for i in range(IC)]
            wv_sb = [persist.tile([P, LC], mdt, tag=f"wv{i}", name=f"wv{i}")
                     for i in range(IC)]
            wp_sb = [persist.tile([P, C], mdt, tag=f"wp{i}", name=f"wp{i}")
                     for i in range(4)]

            xpool = st.enter_context(tc.tile_pool(name="xs", bufs=2))
            apool = st.enter_context(tc.tile_pool(name="att", bufs=3))
            yrpool = st.enter_context(tc.tile_pool(name="yraw", bufs=4))
            ztpool = st.enter_context(tc.tile_pool(name="zev", bufs=3))
            # PSUM: pab 2 + ps 2x2 + po 2 = 8 banks
            pab = st.enter_context(tc.tile_pool(name="pab", bufs=2, space="PSUM"))
            pss = st.enter_context(tc.tile_pool(name="pss", bufs=2, space="PSUM"))
            pso = st.enter_context(tc.tile_pool(name="pso", bufs=2, space="PSUM"))

            for i in range(2):
                nc.vector.memset(sgs[i][:], 1.0)
                nc.vector.memset(E_bc[i][:], 0.0)
            nc.vector.memset(E_bc[0][0:1, 0:D], 1.0)
            nc.vector.memset(E_bc[0][32:33, D:P], 1.0)
            nc.vector.memset(E_bc[1][64:65, 0:D], 1.0)
            nc.vector.memset(E_bc[1][96:97, D:P], 1.0)

            prev = None
            for tt in range(NQT):
                # ---- projections for token tile tt ----
                xt = [xpool.tile([P, QT], mdt, tag=f"x{i}", name=f"x{i}")
                      for i in range(IC)]
                for i in range(IC):
                    nc.sync.dma_start(
                        xt[i][:], xT[i * P:(i + 1) * P, tt * QT:(tt + 1) * QT])
                    if tt == 0:
                        # x/qk weights first so the first matmul starts early
                        nc.sync.dma_start(wqk_sb[i][:],
                                          wqkT[i * P:(i + 1) * P, :])
                if tt == 0:
                    for i in range(IC):
                        nc.sync.dma_start(wv_sb[i][:], wvT[i * P:(i + 1) * P, :])
                    for i in range(4):
                        nc.sync.dma_start(wp_sb[i][:], wpT[i * P:(i + 1) * P, :])
                    nc.sync.dma_start(bqk_sb[:], bqk)
                    nc.sync.dma_start(bpj_sb[:], bpj)
                    nc.sync.dma_start(maskf[:], maskd)
                # qk-proj: psum[out-ch 128, tok 512] accumulated over in-ch
                for oc in range(8):
                    ps = pab.tile([P, QT], f32, tag="pab", name="pab")
                    for i in range(IC):
                        nc.tensor.matmul(
                            ps[:], wqk_sb[i][:, oc * P:(oc + 1) * P],
                            xt[i][:], start=(i == 0), stop=(i == IC - 1))
                    nc.scalar.activation(
                        qk_sb[oc][:, tt * QT:(tt + 1) * QT], ps[:],
                        mybir.ActivationFunctionType.Identity,
                        bias=bqk_sb[:, oc:oc + 1])
                # v-proj: psum[tok 128, out-ch 512] per tok block
                for tb in range(4):
                    kb = tt * 4 + tb
                    ps = pab.tile([P, NH, D], f32, tag="pab", name="pab")
                    for i in range(IC):
                        nc.tensor.matmul(
                            ps[:], xt[i][:, tb * P:(tb + 1) * P],
                            wv_sb[i][:], start=(i == 0), stop=(i == IC - 1))
                    nc.scalar.activation(v_sb[kb][:, :, 0:D], ps[:], Copy)
                    nc.vector.memset(v_sb[kb][:, :, D:D + 1], 1.0)

                def norm_cproj(ptt, yr2p):
                    for j in range(4):
                        # heads 2j,2j+1 live in sg tile j//2 rows 64(j%2)+{0,32}
                        btp = pab.tile([P, QT], f32, tag="pab", name="btp")
                        nc.tensor.matmul(btp[:], E_bc[j % 2][:],
                                         rgs[j // 2][:], start=True, stop=True)
                        nc.vector.tensor_mul(
                            y_sb[j][:, ptt * QT:(ptt + 1) * QT],
                            yr2p[j][:], btp[:])
                    for oc in range(8):
                        ps = pab.tile([P, QT], f32, tag="pab", name="pab")
                        for i in range(4):
                            nc.tensor.matmul(
                                ps[:], wp_sb[i][:, oc * P:(oc + 1) * P],
                                y_sb[i][:, ptt * QT:(ptt + 1) * QT],
                                start=(i == 0), stop=(i == 3))
                        zt = ztpool.tile([P, QT], f32, tag="zt", name="zt")
                        nc.vector.tensor_scalar_add(zt[:], ps[:],
                                                    bpj_sb[:, oc:oc + 1])
                        nc.sync.dma_start(
                            zT[oc * P:(oc + 1) * P,
                               ptt * QT:(ptt + 1) * QT], zt[:])

                if prev is not None:
                    norm_cproj(*prev)

                # ---- attention for query tile tt ----
                qtt = tt
                nkb = (qtt + 1) * 4
                npair = nkb // 2
                # task list: (h, pi); S+exp emitted one slot ahead of att@V
                tasks = [(h, pi) for h in range(NH) for pi in range(npair)]
                po_t = [None] * NH
                at_t = {}
                yr2 = [None] * 4

                def emit_s(h, pi):
                    p0 = (h % 2) * D
                    qt_i = h // 2
                    kt_i = 4 + h // 2
                    kbs = (2 * pi, 2 * pi + 1)
                    ns, c0s = [], []
                    for kb in kbs:
                        e = kb * P - qtt * QT
                        c0s.append(max(e, 0))
                        ns.append(QT - max(e, 0))
                    o1 = ns[0] if ns[0] + ns[1] <= QT else QT
                    width = o1 + ns[1]
                    ps = pss.tile([P, 2 * QT], f32, tag="ps", name="ps")
                    at = apool.tile([P, 2 * QT], mdt, tag="at", name="at")
                    for kb, n, c0, o in zip(kbs, ns, c0s, (0, o1)):
                        nc.tensor.matmul(
                            ps[:, o:o + n],
                            qk_sb[kt_i][p0:p0 + D, kb * P:(kb + 1) * P],
                            qk_sb[qt_i][p0:p0 + D,
                                        qtt * QT + c0:(qtt + 1) * QT],
                            start=True, stop=True)
                    nc.scalar.activation(at[:, 0:width], ps[:, 0:width],
                                         Exp, scale=0.125)
                    for kb, n, c0, o in zip(kbs, ns, c0s, (0, o1)):
                        if kb * P - qtt * QT >= 0:
                            # zero strict upper triangle; never reaches past
                            # the first 128 live columns
                            m = min(n, P)
                            nc.vector.tensor_mul(at[:, o:o + m],
                                                 at[:, o:o + m],
                                                 maskf[:, 0:m])
                    at_t[(h, pi)] = (at, ns, c0s, (0, o1))

                def emit_a(h, pi):
                    at, ns, c0s, os_ = at_t.pop((h, pi))
                    kbs = (2 * pi, 2 * pi + 1)
                    if pi == 0:
                        po_t[h] = pso.tile([D + 1, QT], f32, tag="po", name="po")
                    po = po_t[h]
                    for kb, n, c0, o in zip(kbs, ns, c0s, os_):
                        nc.tensor.matmul(
                            po[:, c0:QT], v_sb[kb][:, h, :], at[:, o:o + n],
                            start=(kb == 0), stop=(kb == nkb - 1))
                    if pi == npair - 1:
                        # evict numerator into the paired-head tile and the
                        # denominator row into its sg slot; frees the bank
                        j = h // 2
                        if yr2[j] is None:
                            yr2[j] = yrpool.tile([P, QT], f32, tag="yr", name="yr")
                        r0 = D * (h % 2)
                        nc.vector.tensor_copy(yr2[j][r0:r0 + D, :], po[0:D, :])
                        nc.vector.tensor_copy(
                            sgs[h // 4][32 * (h % 4):32 * (h % 4) + 1, :],
                            po[D:D + 1, :])
                        po_t[h] = None

                for s, t in enumerate(tasks):
                    emit_s(*t)
                    if s >= 1:
                        emit_a(*tasks[s - 1])
                emit_a(*tasks[-1])

                for g in range(2):
                    nc.vector.reciprocal(rgs[g][:], sgs[g][:])
                prev = (tt, yr2)
            norm_cproj(*prev)
    nc.compile()
    return nc


def get_nc():
    if "nc" not in _nc_cache:
        _nc_cache["nc"] = _build_nc()
    return _nc_cache["nc"]


def _mm_np_dtype():
    if MM_DT == "bfloat16":
        import ml_dtypes
        return np.dtype(ml_dtypes.bfloat16)
    return np.dtype(np.float32)


def make_in_maps(x, Wqkv, bqkv, Wproj, bproj):
    x = np.asarray(x, np.float32)
    Wqkv = np.asarray(Wqkv, np.float32)
    bqkv = np.asarray(bqkv, np.float32)
    Wproj = np.asarray(Wproj, np.float32)
    bproj = np.asarray(bproj, np.float32)
    Wq, Wk, Wv = Wqkv[0:C], Wqkv[C:2 * C], Wqkv[2 * C:3 * C]
    bq, bk, bv = bqkv[0:C], bqkv[C:2 * C], bqkv[2 * C:3 * C]
    mdt = _mm_np_dtype()
    mask = (np.arange(QT)[None, :] >= np.arange(P)[:, None]).astype(mdt)
    in_maps = []
    for b in range(B):
        xTb = np.ascontiguousarray(x[b].T.astype(mdt))
        for s in range(2):
            cols = slice(s * LC, (s + 1) * LC)
            wqkT = np.ascontiguousarray(
                np.concatenate([Wq[cols], Wk[cols]], 0).T.astype(mdt))
            bqk_ = np.concatenate([bq[cols], bk[cols]])
            wvT_ = np.ascontiguousarray(Wv[cols].T.astype(mdt))
            wpT_ = np.ascontiguousarray(Wproj[:, cols].T.astype(mdt))
            bp_eff = bv[cols] @ Wproj[:, cols].T
            if s == 0:
                bp_eff = bp_eff + bproj
            in_maps.append({
                "xT": xTb,
                "wqkT": wqkT,
                "bqk": np.ascontiguousarray(bqk_.reshape(8, P).T),
                "wvT": wvT_,
                "wpT": wpT_,
                "bpj": np.ascontiguousarray(
                    bp_eff.astype(np.float32).reshape(8, P).T),
                "mask": mask,
            })
    return in_maps


def gather_out(results):
    out = np.empty((B, T, C), np.float32)
    for b in range(B):
        zt = results[2 * b]["zT"] + results[2 * b + 1]["zT"]
        out[b] = zt.T
    return out


def kernel(x, Wqkv, bqkv, Wproj, bproj):
    from concourse.bass_utils import run_bass_kernel_spmd

    in_maps = make_in_maps(x, Wqkv, bqkv, Wproj, bproj)
    try:
        res = run_bass_kernel_spmd(get_nc(), in_maps, core_ids=list(range(8)))
    except Exception:
        # transient device faults have been observed once; retry a single time
        res = run_bass_kernel_spmd(get_nc(), in_maps, core_ids=list(range(8)))
    return gather_out(res.results)


# revision 10
# speedup vs baseline: 1.4704x; 1.0129x over previous
"""Causal self-attention (B=4, T=2048, C=1024, H=16) on 8 trn2 NeuronCores.

Sharding: core = (batch b, head-half s).  Each core computes q/k/v
projections for its 8 heads (weights pre-sliced/transposed on host),
causal flash-style attention with transposed score tiles, and a partial
(row-sharded) c_proj.  Host gather sums the two partials per batch.

v4: bf16 matmul inputs; proj/attention merged per token tile; attention
software-pipelined (S-pair emitted one slot ahead of its att@V consumer
so the PE never waits on exp); normalize+cproj of tile tt deferred past
proj(tt+1) so the slow DVE reciprocal never heads the PE queue; host
packs weights/x/z into [128, blk, free] so each tensor is one big DMA.

Device data layout (bf16 unless noted):
  xT    [128, 8, 2048]  x[b].T blocked     xT[p,i,t] = x[b][t, i*128+p]
  wqkT  [128, 8, 1024]  [Wq_l | Wk_l].T blocked
  bqk   [128, 8] f32    q/k bias, per out-ch block
  wvT   [128, 8, 512]   Wv_local.T blocked
  wpT   [128, 4, 1024]  Wproj[:, local].T blocked
  bpj   [128, 8] f32    bproj + bv@WprojT (folded), half per core
  mask  [128, 512]      causal keep-mask m[p,c] = (c >= p)
  zT    [128, 8, 2048] f32  partial out, z[t, oc*128+p] = zT[p,oc,t]
"""

import os
import sys

sys.path.insert(0, "/opt/trn_rl_repo")

import numpy as np

B, T, C, H = 4, 2048, 1024, 16
D = 64          # head dim
NH = 8          # heads per core
LC = NH * D     # local channels = 512
P = 128
QT = 512        # query tile (also matmul moving free dim)
NQT = T // QT   # 4
NKB = T // P    # 16 key blocks
IC = C // P     # 8 input-channel blocks

# matmul input dtype: bfloat16 = full-rate, float32r fallback (exact-ish)
MM_DT = os.environ.get("BASS_ATTN_MM_DT", "bfloat16")

_nc_cache = {}


def _build_nc():
    from contextlib import ExitStack

    import concourse.bass as bass  # noqa: F401
    import concourse.mybir as mybir
    from concourse import bacc, tile

    f32 = mybir.dt.float32
    mdt = getattr(mybir.dt, MM_DT)
    Exp = mybir.ActivationFunctionType.Exp
    Ident = mybir.ActivationFunctionType.Identity
    Copy = mybir.ActivationFunctionType.Copy

    nc = bacc.Bacc("TRN2", target_bir_lowering=False, debug=False, num_devices=8)
    xT = nc.dram_tensor("xT", [P, IC, T], mdt, kind="ExternalInput").ap()
    wqkT = nc.dram_tensor("wqkT", [P, IC, 2 * LC], mdt, kind="ExternalInput").ap()
    bqk = nc.dram_tensor("bqk", [P, 8], f32, kind="ExternalInput").ap()
    wvT = nc.dram_tensor("wvT", [P, IC, LC], mdt, kind="ExternalInput").ap()
    wpT = nc.dram_tensor("wpT", [P, 4, C], mdt, kind="ExternalInput").ap()
    bpj = nc.dram_tensor("bpj", [P, 8], f32, kind="ExternalInput").ap()
    maskd = nc.dram_tensor("mask", [P, QT], mdt, kind="ExternalInput").ap()
    zT = nc.dram_tensor("zT", [P, 8, T], f32, kind="ExternalOutput").ap()

    with tile.TileContext(nc) as tc:
        with ExitStack() as st:
            st.enter_context(nc.allow_low_precision(
                reason="bf16 throughput; accuracy checked vs reference"))
            persist = st.enter_context(tc.tile_pool(name="persist", bufs=1))
            # qk_sb: out-ch blocks 0-3 = q, 4-7 = k; [out-ch 128, tok 2048]
            qk_sb = [persist.tile([P, T], mdt, tag=f"qk{i}", name=f"qk{i}")
                     for i in range(8)]
            # v_sb[kb]: [tok 128, head 8, d 64 + ones col]
            v_sb = [persist.tile([P, NH, D + 1], mdt, tag=f"v{i}", name=f"v{i}")
                    for i in range(NKB)]
            # y_sb: attention out, [local-ch 128, tok 2048] x 4 blocks
            y_sb = [persist.tile([P, T], mdt, tag=f"y{i}", name=f"y{i}")
                    for i in range(4)]
            bqk_sb = persist.tile([P, 8], f32, tag="bqk", name="bqk")
            bpj_sb = persist.tile([P, 8], f32, tag="bpj", name="bpj")
            maskf = persist.tile([P, QT], mdt, tag="maskf", name="maskf")
            # softmax-sum rows: 4 heads per tile at 32-aligned partitions;
            # memset once so dead rows never produce inf/NaN via reciprocal
            sgs = [persist.tile([P, QT], f32, tag=f"sg{i}", name=f"sg{i}")
                   for i in range(2)]
            rgs = [persist.tile([P, QT], mdt, tag=f"rg{i}", name=f"rg{i}")
                   for i in range(2)]
            # E matrices: block row-broadcast for paired-head normalization.
            # btp[p,c] = sum_k E[k,p]*rg[k,c]; E_lo rows {0,32}, E_hi {64,96}
            E_bc = [persist.tile([P, P], mdt, tag=f"E{i}", name=f"E{i}")
                    for i in range(2)]
            # weights, host-packed [128, blk, free]
            wqk_sb = persist.tile([P, IC, 2 * LC], mdt, tag="wqk", name="wqk")
            wv_sb = persist.tile([P, IC, LC], mdt, tag="wv", name="wv")
            wp_sb = persist.tile([P, 4, C], mdt, tag="wp", name="wp")

            xpool = st.enter_context(tc.tile_pool(name="xs", bufs=2))
            apool = st.enter_context(tc.tile_pool(name="att", bufs=3))
            yrpool = st.enter_context(tc.tile_pool(name="yraw", bufs=4))
            ztpool = st.enter_context(tc.tile_pool(name="zev", bufs=2))
            # PSUM: pab 2 + ps 2x2 + po 2 = 8 banks
            pab = st.enter_context(tc.tile_pool(name="pab", bufs=2, space="PSUM"))
            pss = st.enter_context(tc.tile_pool(name="pss", bufs=2, space="PSUM"))
            pso = st.enter_context(tc.tile_pool(name="pso", bufs=2, space="PSUM"))

            for i in range(2):
                nc.vector.memset(sgs[i][:], 1.0)
                nc.vector.memset(E_bc[i][:], 0.0)
            nc.vector.memset(E_bc[0][0:1, 0:D], 1.0)
            nc.vector.memset(E_bc[0][32:33, D:P], 1.0)
            nc.vector.memset(E_bc[1][64:65, 0:D], 1.0)
            nc.vector.memset(E_bc[1][96:97, D:P], 1.0)

            def btp_norm(j, ptt, yr2p):
                # heads 2j,2j+1 live in rg tile j//2 rows 64(j%2)+{0,32}
                btp = pab.tile([P, QT], f32, tag="pab", name="btp")
                nc.tensor.matmul(btp[:], E_bc[j % 2][:], rgs[j // 2][:],
                                 start=True, stop=True)
                nc.vector.tensor_mul(
                    y_sb[j][:, ptt * QT:(ptt + 1) * QT], yr2p[j][:], btp[:])

            def cproj_emit(ptt):
                ztile = ztpool.tile([P, 8, QT], f32, tag="zt", name="zt")
                for oc in range(8):
                    ps = pab.tile([P, QT], f32, tag="pab", name="pab")
                    for i in range(4):
                        nc.tensor.matmul(
                            ps[:], wp_sb[:, i, oc * P:(oc + 1) * P],
                            y_sb[i][:, ptt * QT:(ptt + 1) * QT],
                            start=(i == 0), stop=(i == 3))
                    nc.vector.tensor_scalar_add(ztile[:, oc, :], ps[:],
                                                bpj_sb[:, oc:oc + 1])
                nc.sync.dma_start(zT[:, :, ptt * QT:(ptt + 1) * QT], ztile[:])

            prev = None
            for tt in range(NQT):
                # ---- projections for token tile tt ----
                xt = xpool.tile([P, IC, QT], mdt, tag="x", name="x")
                nc.sync.dma_start(xt[:], xT[:, :, tt * QT:(tt + 1) * QT])
                if tt == 0:
                    nc.sync.dma_start(wqk_sb[:], wqkT)
                    nc.sync.dma_start(wv_sb[:], wvT)
                    nc.sync.dma_start(bqk_sb[:], bqk)
                    nc.sync.dma_start(maskf[:], maskd)
                    nc.sync.dma_start(wp_sb[:], wpT)
                    nc.sync.dma_start(bpj_sb[:], bpj)
                # qk-proj: psum[out-ch 128, tok 512] accumulated over in-ch
                for oc in range(8):
                    ps = pab.tile([P, QT], f32, tag="pab", name="pab")
                    for i in range(IC):
                        nc.tensor.matmul(
                            ps[:], wqk_sb[:, i, oc * P:(oc + 1) * P],
                            xt[:, i, :], start=(i == 0), stop=(i == IC - 1))
                    nc.scalar.activation(
                        qk_sb[oc][:, tt * QT:(tt + 1) * QT], ps[:], Ident,
                        bias=bqk_sb[:, oc:oc + 1])
                # v-proj: psum[tok 128, out-ch 512] per tok block
                for tb in range(4):
                    kb = tt * 4 + tb
                    ps = pab.tile([P, NH, D], f32, tag="pab", name="pab")
                    for i in range(IC):
                        nc.tensor.matmul(
                            ps[:], xt[:, i, tb * P:(tb + 1) * P],
                            wv_sb[:, i, :], start=(i == 0), stop=(i == IC - 1))
                    nc.scalar.activation(v_sb[kb][:, :, 0:D], ps[:], Copy)
                    nc.vector.memset(v_sb[kb][:, :, D:D + 1], 1.0)

                # deferred normalize + c_proj for the previous token tile:
                # runs while this tile's attention streams, so the PE queue
                # never stalls behind the reciprocal
                if prev is not None:
                    for j in range(4):
                        btp_norm(j, *prev)
                    cproj_emit(prev[0])

                # ---- attention for query tile tt ----
                qtt = tt
                nkb = (qtt + 1) * 4
                npair = nkb // 2
                # task list: (h, pi); S+exp emitted one slot ahead of att@V
                tasks = [(h, pi) for h in range(NH) for pi in range(npair)]
                po_t = [None] * NH
                at_t = {}
                yr2 = [None] * 4

                def emit_s(h, pi):
                    p0 = (h % 2) * D
                    qt_i = h // 2
                    kt_i = 4 + h // 2
                    kbs = (2 * pi, 2 * pi + 1)
                    ns, c0s = [], []
                    for kb in kbs:
                        e = kb * P - qtt * QT
                        c0s.append(max(e, 0))
                        ns.append(QT - max(e, 0))
                    o1 = ns[0] if ns[0] + ns[1] <= QT else QT
                    width = o1 + ns[1]
                    ps = pss.tile([P, 2 * QT], f32, tag="ps", name="ps")
                    at = apool.tile([P, 2 * QT], mdt, tag="at", name="at")
                    for kb, n, c0, o in zip(kbs, ns, c0s, (0, o1)):
                        nc.tensor.matmul(
                            ps[:, o:o + n],
                            qk_sb[kt_i][p0:p0 + D, kb * P:(kb + 1) * P],
                            qk_sb[qt_i][p0:p0 + D,
                                        qtt * QT + c0:(qtt + 1) * QT],
                            start=True, stop=True)
                    nc.scalar.activation(at[:, 0:width], ps[:, 0:width],
                                         Exp, scale=0.125)
                    for kb, n, c0, o in zip(kbs, ns, c0s, (0, o1)):
                        if kb * P - qtt * QT >= 0:
                            # zero strict upper triangle; never reaches past
                            # the first 128 live columns
                            m = min(n, P)
                            nc.vector.tensor_mul(at[:, o:o + m],
                                                 at[:, o:o + m],
                                                 maskf[:, 0:m])
                    at_t[(h, pi)] = (at, ns, c0s, (0, o1))

                def emit_a(h, pi):
                    at, ns, c0s, os_ = at_t.pop((h, pi))
                    kbs = (2 * pi, 2 * pi + 1)
                    if pi == 0:
                        po_t[h] = pso.tile([D + 1, QT], f32, tag="po",
                                           name="po")
                    po = po_t[h]
                    for kb, n, c0, o in zip(kbs, ns, c0s, os_):
                        nc.tensor.matmul(
                            po[:, c0:QT], v_sb[kb][:, h, :], at[:, o:o + n],
                            start=(kb == 0), stop=(kb == nkb - 1))
                    if pi == npair - 1:
                        # evict numerator into the paired-head tile and the
                        # denominator row into its sg slot; frees the bank
                        j = h // 2
                        if yr2[j] is None:
                            yr2[j] = yrpool.tile([P, QT], f32, tag="yr",
                                                 name="yr")
                        r0 = D * (h % 2)
                        nc.vector.tensor_copy(yr2[j][r0:r0 + D, :], po[0:D, :])
                        nc.vector.tensor_copy(
                            sgs[h // 4][32 * (h % 4):32 * (h % 4) + 1, :],
                            po[D:D + 1, :])
                        po_t[h] = None

                last = NQT - 1
                for s, t in enumerate(tasks):
                    emit_s(*t)
                    if s >= 1:
                        emit_a(*tasks[s - 1])
                        if tt == last and tasks[s - 1] == (3, npair - 1):
                            # final tile: normalize heads 0-3 mid-stream so
                            # only half the chain lands on the kernel tail
                            nc.vector.reciprocal(rgs[0][:], sgs[0][:])
                            btp_norm(0, tt, yr2)
                            btp_norm(1, tt, yr2)
                emit_a(*tasks[-1])
                if tt == last:
                    nc.vector.reciprocal(rgs[1][:], sgs[1][:])
                    btp_norm(2, tt, yr2)
                    btp_norm(3, tt, yr2)
                    cproj_emit(tt)
                else:
                    for g in range(2):
                        nc.vector.reciprocal(rgs[g][:], sgs[g][:])
                    prev = (tt, yr2)
    nc.compile()
    return nc


def get_nc():
    if "nc" not in _nc_cache:
        _nc_cache["nc"] = _build_nc()
    return _nc_cache["nc"]


def _mm_np_dtype():
    if MM_DT == "bfloat16":
        import ml_dtypes
        return np.dtype(ml_dtypes.bfloat16)
    return np.dtype(np.float32)


def _blk(a, nb):
    """[nb*128, F] -> [128, nb, F] with out[p, i, f] = a[i*128+p, f]."""
    return np.ascontiguousarray(
        a.reshape(nb, P, -1).transpose(1, 0, 2))


def make_in_maps(x, Wqkv, bqkv, Wproj, bproj):
    x = np.asarray(x, np.float32)
    Wqkv = np.asarray(Wqkv, np.float32)
    bqkv = np.asarray(bqkv, np.float32)
    Wproj = np.asarray(Wproj, np.float32)
    bproj = np.asarray(bproj, np.float32)
    Wq, Wk, Wv = Wqkv[0:C], Wqkv[C:2 * C], Wqkv[2 * C:3 * C]
    bq, bk, bv = bqkv[0:C], bqkv[C:2 * C], bqkv[2 * C:3 * C]
    mdt = _mm_np_dtype()
    mask = (np.arange(QT)[None, :] >= np.arange(P)[:, None]).astype(mdt)
    in_maps = []
    for b in range(B):
        xTb = _blk(x[b].T.astype(mdt), IC)
        for s in range(2):
            cols = slice(s * LC, (s + 1) * LC)
            wqkT = _blk(np.concatenate(
                [Wq[cols], Wk[cols]], 0).T.astype(mdt), IC)
            bqk_ = np.concatenate([bq[cols], bk[cols]])
            wvT_ = _blk(Wv[cols].T.astype(mdt), IC)
            wpT_ = _blk(Wproj[:, cols].T.astype(mdt), 4)
            bp_eff = bv[cols] @ Wproj[:, cols].T
            if s == 0:
                bp_eff = bp_eff + bproj
            in_maps.append({
                "xT": xTb,
                "wqkT": wqkT,
                "bqk": np.ascontiguousarray(bqk_.reshape(8, P).T),
                "wvT": wvT_,
                "wpT": wpT_,
                "bpj": np.ascontiguousarray(
                    bp_eff.astype(np.float32).reshape(8, P).T),
                "mask": mask,
            })
    return in_maps


def gather_out(results):
    out = np.empty((B, T, C), np.float32)
    for b in range(B):
        zt = results[2 * b]["zT"] + results[2 * b + 1]["zT"]
        # zt[p, oc, t] -> z[t, oc*128+p]
        out[b] = zt.transpose(1, 0, 2).reshape(C, T).T
    return out


def kernel(x, Wqkv, bqkv, Wproj, bproj):
    from concourse.bass_utils import run_bass_kernel_spmd

    in_maps = make_in_maps(x, Wqkv, bqkv, Wproj, bproj)
    try:
        res = run_bass_kernel_spmd(get_nc(), in_maps, core_ids=list(range(8)))
    except Exception:
        # transient device faults have been observed once; retry a single time
        res = run_bass_kernel_spmd(get_nc(), in_maps, core_ids=list(range(8)))
    return gather_out(res.results)


# revision 11
# speedup vs baseline: 1.5518x; 1.0554x over previous
"""Causal self-attention (B=4, T=2048, C=1024, H=16) on 8 trn2 NeuronCores.

Sharding: core = (batch b, head-half s).  Each core computes q/k/v
projections for its 8 heads (weights pre-sliced/transposed on host),
causal flash-style attention with transposed score tiles, and a partial
(row-sharded) c_proj.  Host gather sums the two partials per batch.

v4: bf16 matmul inputs; proj/attention merged per token tile; attention
software-pipelined (S-pair emitted one slot ahead of its att@V consumer
so the PE never waits on exp); normalize+cproj of tile tt deferred past
proj(tt+1) so the slow DVE reciprocal never heads the PE queue; host
packs weights/x/z into [128, blk, free] so each tensor is one big DMA.

Device data layout (bf16 unless noted):
  xT    [128, 8, 2048]  x[b].T blocked     xT[p,i,t] = x[b][t, i*128+p]
  wqkT  [128, 8, 1024]  [Wq_l | Wk_l].T blocked
  bqk   [128, 8] f32    q/k bias, per out-ch block
  wvT   [128, 8, 512]   Wv_local.T blocked
  wpT   [128, 4, 1024]  Wproj[:, local].T blocked
  bpj   [128, 8] f32    bproj + bv@WprojT (folded), half per core
  mask  [128, 512]      causal keep-mask m[p,c] = (c >= p)
  zT    [128, 8, 2048] f32  partial out, z[t, oc*128+p] = zT[p,oc,t]
"""

import os
import sys

sys.path.insert(0, "/opt/trn_rl_repo")

import numpy as np

B, T, C, H = 4, 2048, 1024, 16
D = 64          # head dim
NH = 8          # heads per core
LC = NH * D     # local channels = 512
P = 128
QT = 512        # query tile (also matmul moving free dim)
NQT = T // QT   # 4
NKB = T // P    # 16 key blocks
IC = C // P     # 8 input-channel blocks

# matmul input dtype: bfloat16 = full-rate, float32r fallback (exact-ish)
MM_DT = os.environ.get("BASS_ATTN_MM_DT", "bfloat16")

_nc_cache = {}


def _build_nc():
    from contextlib import ExitStack

    import concourse.bass as bass  # noqa: F401
    import concourse.mybir as mybir
    from concourse import bacc, tile

    f32 = mybir.dt.float32
    mdt = getattr(mybir.dt, MM_DT)
    Exp = mybir.ActivationFunctionType.Exp
    Ident = mybir.ActivationFunctionType.Identity
    Copy = mybir.ActivationFunctionType.Copy

    nc = bacc.Bacc("TRN2", target_bir_lowering=False, debug=False, num_devices=8)
    xT = nc.dram_tensor("xT", [P, IC, T], mdt, kind="ExternalInput").ap()
    wqkT = nc.dram_tensor("wqkT", [P, IC, 2 * LC], mdt, kind="ExternalInput").ap()
    bqk = nc.dram_tensor("bqk", [P, 8], f32, kind="ExternalInput").ap()
    wvT = nc.dram_tensor("wvT", [P, IC, LC], mdt, kind="ExternalInput").ap()
    wpT = nc.dram_tensor("wpT", [P, 4, C], mdt, kind="ExternalInput").ap()
    bpj = nc.dram_tensor("bpj", [P, 8], f32, kind="ExternalInput").ap()
    maskd = nc.dram_tensor("mask", [P, QT], mdt, kind="ExternalInput").ap()
    zT = nc.dram_tensor("zT", [P, 8, T], f32, kind="ExternalOutput").ap()

    with tile.TileContext(nc) as tc:
        with ExitStack() as st:
            st.enter_context(nc.allow_low_precision(
                reason="bf16 throughput; accuracy checked vs reference"))
            persist = st.enter_context(tc.tile_pool(name="persist", bufs=1))
            # qk_sb: out-ch blocks 0-3 = q, 4-7 = k; [out-ch 128, tok 2048]
            qk_sb = [persist.tile([P, T], mdt, tag=f"qk{i}", name=f"qk{i}")
                     for i in range(8)]
            # v_sb[kb]: [tok 128, head 8, d 64 + ones col]
            v_sb = [persist.tile([P, NH, D + 1], mdt, tag=f"v{i}", name=f"v{i}")
                    for i in range(NKB)]
            # y_sb: attention out, [local-ch 128, tok 2048] x 4 blocks
            y_sb = [persist.tile([P, T], mdt, tag=f"y{i}", name=f"y{i}")
                    for i in range(4)]
            bqk_sb = persist.tile([P, 8], f32, tag="bqk", name="bqk")
            bpj_sb = persist.tile([P, 8], f32, tag="bpj", name="bpj")
            maskf = persist.tile([P, QT], mdt, tag="maskf", name="maskf")
            # softmax-sum rows: 4 heads per tile at 32-aligned partitions;
            # memset once so dead rows never produce inf/NaN via reciprocal
            sgs = [persist.tile([P, QT], f32, tag=f"sg{i}", name=f"sg{i}")
                   for i in range(2)]
            rgs = [persist.tile([P, QT], mdt, tag=f"rg{i}", name=f"rg{i}")
                   for i in range(2)]
            # E matrices: block row-broadcast for paired-head normalization.
            # btp[p,c] = sum_k E[k,p]*rg[k,c]; E_lo rows {0,32}, E_hi {64,96}
            E_bc = [persist.tile([P, P], mdt, tag=f"E{i}", name=f"E{i}")
                    for i in range(2)]
            # weights, host-packed [128, blk, free]
            wqk_sb = persist.tile([P, IC, 2 * LC], mdt, tag="wqk", name="wqk")
            wv_sb = persist.tile([P, IC, LC], mdt, tag="wv", name="wv")
            wp_sb = persist.tile([P, 4, C], mdt, tag="wp", name="wp")

            xpool = st.enter_context(tc.tile_pool(name="xs", bufs=2))
            apool = st.enter_context(tc.tile_pool(name="att", bufs=3))
            yrpool = st.enter_context(tc.tile_pool(name="yraw", bufs=4))
            ztpool = st.enter_context(tc.tile_pool(name="zev", bufs=2))
            # PSUM: pab 2 + ps 2x2 + po 2 = 8 banks
            pab = st.enter_context(tc.tile_pool(name="pab", bufs=2, space="PSUM"))
            pss = st.enter_context(tc.tile_pool(name="pss", bufs=2, space="PSUM"))
            pso = st.enter_context(tc.tile_pool(name="pso", bufs=2, space="PSUM"))

            for i in range(2):
                nc.vector.memset(sgs[i][:], 1.0)
                nc.vector.memset(E_bc[i][:], 0.0)
            nc.vector.memset(E_bc[0][0:1, 0:D], 1.0)
            nc.vector.memset(E_bc[0][32:33, D:P], 1.0)
            nc.vector.memset(E_bc[1][64:65, 0:D], 1.0)
            nc.vector.memset(E_bc[1][96:97, D:P], 1.0)

            def btp_norm(j, ptt, yr2p):
                # heads 2j,2j+1 live in rg tile j//2 rows 64(j%2)+{0,32}
                btp = pab.tile([P, QT], f32, tag="pab", name="btp")
                nc.tensor.matmul(btp[:], E_bc[j % 2][:], rgs[j // 2][:],
                                 start=True, stop=True)
                nc.vector.tensor_mul(
                    y_sb[j][:, ptt * QT:(ptt + 1) * QT], yr2p[j][:], btp[:])

            def cproj_chunks(ptt):
                # one thunk per output-channel block; interleaved into the
                # attention stream as PE gap-filler so the HAM clock gate
                # never sees an idle window during ACT-bound stretches
                ztile = ztpool.tile([P, 8, QT], f32, tag="zt", name="zt")

                def mk(oc):
                    def go():
                        ps = pab.tile([P, QT], f32, tag="pab", name="pab")
                        for i in range(4):
                            nc.tensor.matmul(
                                ps[:], wp_sb[:, i, oc * P:(oc + 1) * P],
                                y_sb[i][:, ptt * QT:(ptt + 1) * QT],
                                start=(i == 0), stop=(i == 3))
                        nc.vector.tensor_scalar_add(ztile[:, oc, :], ps[:],
                                                    bpj_sb[:, oc:oc + 1])
                        if oc == 7:
                            nc.sync.dma_start(
                                zT[:, :, ptt * QT:(ptt + 1) * QT], ztile[:])
                    return go
                return [mk(oc) for oc in range(8)]

            prev = None
            for tt in range(NQT):
                # ---- projections for token tile tt ----
                xt = xpool.tile([P, IC, QT], mdt, tag="x", name="x")
                if tt == 0:
                    # halves so the first qk chain starts after ~1.5MB
                    h = IC // 2
                    nc.sync.dma_start(xt[:, 0:h, :],
                                      xT[:, 0:h, 0:QT])
                    nc.sync.dma_start(wqk_sb[:, 0:h, :], wqkT[:, 0:h, :])
                    nc.sync.dma_start(xt[:, h:IC, :],
                                      xT[:, h:IC, 0:QT])
                    nc.sync.dma_start(wqk_sb[:, h:IC, :], wqkT[:, h:IC, :])
                    nc.sync.dma_start(bqk_sb[:], bqk)
                    nc.sync.dma_start(wv_sb[:], wvT)
                    nc.sync.dma_start(maskf[:], maskd)
                    nc.sync.dma_start(wp_sb[:], wpT)
                    nc.sync.dma_start(bpj_sb[:], bpj)
                else:
                    nc.sync.dma_start(xt[:], xT[:, :, tt * QT:(tt + 1) * QT])
                # qk-proj: psum[out-ch 128, tok 512] accumulated over in-ch
                for oc in range(8):
                    ps = pab.tile([P, QT], f32, tag="pab", name="pab")
                    for i in range(IC):
                        nc.tensor.matmul(
                            ps[:], wqk_sb[:, i, oc * P:(oc + 1) * P],
                            xt[:, i, :], start=(i == 0), stop=(i == IC - 1))
                    nc.scalar.activation(
                        qk_sb[oc][:, tt * QT:(tt + 1) * QT], ps[:], Ident,
                        bias=bqk_sb[:, oc:oc + 1])
                # v-proj: psum[tok 128, out-ch 512] per tok block
                for tb in range(4):
                    kb = tt * 4 + tb
                    ps = pab.tile([P, NH, D], f32, tag="pab", name="pab")
                    for i in range(IC):
                        nc.tensor.matmul(
                            ps[:], xt[:, i, tb * P:(tb + 1) * P],
                            wv_sb[:, i, :], start=(i == 0), stop=(i == IC - 1))
                    nc.scalar.activation(v_sb[kb][:, :, 0:D], ps[:], Copy)
                    nc.vector.memset(v_sb[kb][:, :, D:D + 1], 1.0)

                # deferred normalize + c_proj for the previous token tile:
                # runs while this tile's attention streams, so the PE queue
                # never stalls behind the reciprocal
                chunks = []
                if prev is not None:
                    for j in range(4):
                        btp_norm(j, *prev)
                    chunks = cproj_chunks(prev[0])

                # ---- attention for query tile tt ----
                qtt = tt
                nkb = (qtt + 1) * 4
                npair = nkb // 2
                # task list: (h, pi); S+exp emitted one slot ahead of att@V
                tasks = [(h, pi) for h in range(NH) for pi in range(npair)]
                po_t = [None] * NH
                at_t = {}
                yr2 = [None] * 4

                def emit_s(h, pi):
                    p0 = (h % 2) * D
                    qt_i = h // 2
                    kt_i = 4 + h // 2
                    kbs = (2 * pi, 2 * pi + 1)
                    ns, c0s = [], []
                    for kb in kbs:
                        e = kb * P - qtt * QT
                        c0s.append(max(e, 0))
                        ns.append(QT - max(e, 0))
                    o1 = ns[0] if ns[0] + ns[1] <= QT else QT
                    width = o1 + ns[1]
                    ps = pss.tile([P, 2 * QT], f32, tag="ps", name="ps")
                    at = apool.tile([P, 2 * QT], mdt, tag="at", name="at")
                    for kb, n, c0, o in zip(kbs, ns, c0s, (0, o1)):
                        nc.tensor.matmul(
                            ps[:, o:o + n],
                            qk_sb[kt_i][p0:p0 + D, kb * P:(kb + 1) * P],
                            qk_sb[qt_i][p0:p0 + D,
                                        qtt * QT + c0:(qtt + 1) * QT],
                            start=True, stop=True)
                    nc.scalar.activation(at[:, 0:width], ps[:, 0:width],
                                         Exp, scale=0.125)
                    for kb, n, c0, o in zip(kbs, ns, c0s, (0, o1)):
                        if kb * P - qtt * QT >= 0:
                            # zero strict upper triangle; never reaches past
                            # the first 128 live columns
                            m = min(n, P)
                            nc.vector.tensor_mul(at[:, o:o + m],
                                                 at[:, o:o + m],
                                                 maskf[:, 0:m])
                    at_t[(h, pi)] = (at, ns, c0s, (0, o1))

                def emit_a(h, pi):
                    at, ns, c0s, os_ = at_t.pop((h, pi))
                    kbs = (2 * pi, 2 * pi + 1)
                    if pi == 0:
                        po_t[h] = pso.tile([D + 1, QT], f32, tag="po",
                                           name="po")
                    po = po_t[h]
                    for kb, n, c0, o in zip(kbs, ns, c0s, os_):
                        nc.tensor.matmul(
                            po[:, c0:QT], v_sb[kb][:, h, :], at[:, o:o + n],
                            start=(kb == 0), stop=(kb == nkb - 1))
                    if pi == npair - 1:
                        # evict numerator into the paired-head tile and the
                        # denominator row into its sg slot; frees the bank
                        j = h // 2
                        if yr2[j] is None:
                            yr2[j] = yrpool.tile([P, QT], f32, tag="yr",
                                                 name="yr")
                        r0 = D * (h % 2)
                        nc.vector.tensor_copy(yr2[j][r0:r0 + D, :], po[0:D, :])
                        nc.vector.tensor_copy(
                            sgs[h // 4][32 * (h % 4):32 * (h % 4) + 1, :],
                            po[D:D + 1, :])
                        po_t[h] = None

                last = NQT - 1
                stride = max(2, len(tasks) // max(len(chunks), 1))
                ci = 0
                for s, t in enumerate(tasks):
                    emit_s(*t)
                    if s >= 1:
                        emit_a(*tasks[s - 1])
                        if ci < len(chunks) and s % stride == stride - 1:
                            chunks[ci]()
                            ci += 1
                        if tt == last and tasks[s - 1] == (3, npair - 1):
                            # final tile: normalize heads 0-3 mid-stream so
                            # only half the chain lands on the kernel tail
                            nc.vector.reciprocal(rgs[0][:], sgs[0][:])
                            btp_norm(0, tt, yr2)
                            btp_norm(1, tt, yr2)
                emit_a(*tasks[-1])
                while ci < len(chunks):
                    chunks[ci]()
                    ci += 1
                if tt == last:
                    nc.vector.reciprocal(rgs[1][:], sgs[1][:])
                    btp_norm(2, tt, yr2)
                    btp_norm(3, tt, yr2)
                    for go in cproj_chunks(tt):
                        go()
                else:
                    for g in range(2):
                        nc.vector.reciprocal(rgs[g][:], sgs[g][:])
                    prev = (tt, yr2)
    nc.compile()
    return nc


def get_nc():
    if "nc" not in _nc_cache:
        _nc_cache["nc"] = _build_nc()
    return _nc_cache["nc"]


def _mm_np_dtype():
    if MM_DT == "bfloat16":
        import ml_dtypes
        return np.dtype(ml_dtypes.bfloat16)
    return np.dtype(np.float32)


def _blk(a, nb):
    """[nb*128, F] -> [128, nb, F] with out[p, i, f] = a[i*128+p, f]."""
    return np.ascontiguousarray(
        a.reshape(nb, P, -1).transpose(1, 0, 2))


def make_in_maps(x, Wqkv, bqkv, Wproj, bproj):
    x = np.asarray(x, np.float32)
    Wqkv = np.asarray(Wqkv, np.float32)
    bqkv = np.asarray(bqkv, np.float32)
    Wproj = np.asarray(Wproj, np.float32)
    bproj = np.asarray(bproj, np.float32)
    Wq, Wk, Wv = Wqkv[0:C], Wqkv[C:2 * C], Wqkv[2 * C:3 * C]
    bq, bk, bv = bqkv[0:C], bqkv[C:2 * C], bqkv[2 * C:3 * C]
    mdt = _mm_np_dtype()
    mask = (np.arange(QT)[None, :] >= np.arange(P)[:, None]).astype(mdt)
    in_maps = []
    for b in range(B):
        xTb = _blk(x[b].T.astype(mdt), IC)
        for s in range(2):
            cols = slice(s * LC, (s + 1) * LC)
            wqkT = _blk(np.concatenate(
                [Wq[cols], Wk[cols]], 0).T.astype(mdt), IC)
            bqk_ = np.concatenate([bq[cols], bk[cols]])
            wvT_ = _blk(Wv[cols].T.astype(mdt), IC)
            wpT_ = _blk(Wproj[:, cols].T.astype(mdt), 4)
            bp_eff = bv[cols] @ Wproj[:, cols].T
            if s == 0:
                bp_eff = bp_eff + bproj
            in_maps.append({
                "xT": xTb,
                "wqkT": wqkT,
                "bqk": np.ascontiguousarray(bqk_.reshape(8, P).T),
                "wvT": wvT_,
                "wpT": wpT_,
                "bpj": np.ascontiguousarray(
                    bp_eff.astype(np.float32).reshape(8, P).T),
                "mask": mask,
            })
    return in_maps


def gather_out(results):
    out = np.empty((B, T, C), np.float32)
    for b in range(B):
        zt = results[2 * b]["zT"] + results[2 * b + 1]["zT"]
        # zt[p, oc, t] -> z[t, oc*128+p]
        out[b] = zt.transpose(1, 0, 2).reshape(C, T).T
    return out


def kernel(x, Wqkv, bqkv, Wproj, bproj):
    from concourse.bass_utils import run_bass_kernel_spmd

    in_maps = make_in_maps(x, Wqkv, bqkv, Wproj, bproj)
    try:
        res = run_bass_kernel_spmd(get_nc(), in_maps, core_ids=list(range(8)))
    except Exception:
        # transient device faults have been observed once; retry a single time
        res = run_bass_kernel_spmd(get_nc(), in_maps, core_ids=list(range(8)))
    return gather_out(res.results)


# revision 12
# speedup vs baseline: 1.5886x; 1.0237x over previous
"""Causal self-attention (B=4, T=2048, C=1024, H=16) on 8 trn2 NeuronCores.

Sharding: core = (batch b, head-half s).  Each core computes q/k/v
projections for its 8 heads (weights pre-sliced/transposed on host),
causal flash-style attention with transposed score tiles, and a partial
(row-sharded) c_proj.  Host gather sums the two partials per batch.

v4: bf16 matmul inputs; proj/attention merged per token tile; attention
software-pipelined (S-pair emitted one slot ahead of its att@V consumer
so the PE never waits on exp); normalize+cproj of tile tt deferred past
proj(tt+1) so the slow DVE reciprocal never heads the PE queue; host
packs weights/x/z into [128, blk, free] so each tensor is one big DMA.

Device data layout (bf16 unless noted):
  xT    [128, 8, 2048]  x[b].T blocked     xT[p,i,t] = x[b][t, i*128+p]
  wqkT  [128, 8, 1024]  [Wq_l | Wk_l].T blocked
  bqk   [128, 8] f32    q/k bias, per out-ch block
  wvT   [128, 8, 512]   Wv_local.T blocked
  wpT   [128, 4, 1024]  Wproj[:, local].T blocked
  bpj   [128, 8] f32    bproj + bv@WprojT (folded), half per core
  mask  [128, 512]      causal keep-mask m[p,c] = (c >= p)
  zT    [128, 8, 2048] f32  partial out, z[t, oc*128+p] = zT[p,oc,t]
"""

import os
import sys

sys.path.insert(0, "/opt/trn_rl_repo")

import numpy as np

B, T, C, H = 4, 2048, 1024, 16
D = 64          # head dim
NH = 8          # heads per core
LC = NH * D     # local channels = 512
P = 128
QT = 512        # query tile (also matmul moving free dim)
NQT = T // QT   # 4
NKB = T // P    # 16 key blocks
IC = C // P     # 8 input-channel blocks

# matmul input dtype: bfloat16 = full-rate, float32r fallback (exact-ish)
MM_DT = os.environ.get("BASS_ATTN_MM_DT", "bfloat16")

_nc_cache = {}


def _build_nc():
    from contextlib import ExitStack

    import concourse.bass as bass  # noqa: F401
    import concourse.mybir as mybir
    from concourse import bacc, tile

    f32 = mybir.dt.float32
    mdt = getattr(mybir.dt, MM_DT)
    Exp = mybir.ActivationFunctionType.Exp
    Ident = mybir.ActivationFunctionType.Identity
    Copy = mybir.ActivationFunctionType.Copy

    nc = bacc.Bacc("TRN2", target_bir_lowering=False, debug=False, num_devices=8)
    xT = nc.dram_tensor("xT", [P, IC, T], mdt, kind="ExternalInput").ap()
    wqkT = nc.dram_tensor("wqkT", [P, IC, 2 * LC], mdt, kind="ExternalInput").ap()
    bqk = nc.dram_tensor("bqk", [P, 8], f32, kind="ExternalInput").ap()
    wvT = nc.dram_tensor("wvT", [P, IC, LC], mdt, kind="ExternalInput").ap()
    wpT = nc.dram_tensor("wpT", [P, 4, C], mdt, kind="ExternalInput").ap()
    bpj = nc.dram_tensor("bpj", [P, 8], f32, kind="ExternalInput").ap()
    maskd = nc.dram_tensor("mask", [P, QT], mdt, kind="ExternalInput").ap()
    zT = nc.dram_tensor("zT", [P, 8, T], f32, kind="ExternalOutput").ap()

    with tile.TileContext(nc) as tc:
        with ExitStack() as st:
            st.enter_context(nc.allow_low_precision(
                reason="bf16 throughput; accuracy checked vs reference"))
            persist = st.enter_context(tc.tile_pool(name="persist", bufs=1))
            # qk_sb: out-ch blocks 0-3 = q, 4-7 = k; [out-ch 128, tok 2048]
            qk_sb = [persist.tile([P, T], mdt, tag=f"qk{i}", name=f"qk{i}")
                     for i in range(8)]
            # v_sb[kb]: [tok 128, head 8, d 64 + ones col]
            v_sb = [persist.tile([P, NH, D + 1], mdt, tag=f"v{i}", name=f"v{i}")
                    for i in range(NKB)]
            # y_sb: attention out, [local-ch 128, tok 2048] x 4 blocks
            y_sb = [persist.tile([P, T], mdt, tag=f"y{i}", name=f"y{i}")
                    for i in range(4)]
            bqk_sb = persist.tile([P, 8], f32, tag="bqk", name="bqk")
            bpj_sb = persist.tile([P, 8], f32, tag="bpj", name="bpj")
            maskf = persist.tile([P, QT], mdt, tag="maskf", name="maskf")
            # softmax-sum rows: 4 heads per tile at 32-aligned partitions;
            # memset once so dead rows never produce inf/NaN via reciprocal
            sgs = [persist.tile([P, QT], f32, tag=f"sg{i}", name=f"sg{i}")
                   for i in range(2)]
            rgs = [persist.tile([P, QT], mdt, tag=f"rg{i}", name=f"rg{i}")
                   for i in range(2)]
            # E matrices: block row-broadcast for paired-head normalization.
            # btp[p,c] = sum_k E[k,p]*rg[k,c]; E_lo rows {0,32}, E_hi {64,96}
            E_bc = [persist.tile([P, P], mdt, tag=f"E{i}", name=f"E{i}")
                    for i in range(2)]
            # weights, host-packed [128, blk, free]
            wqk_sb = persist.tile([P, IC, 2 * LC], mdt, tag="wqk", name="wqk")
            wv_sb = persist.tile([P, IC, LC], mdt, tag="wv", name="wv")
            wp_sb = persist.tile([P, 4, C], mdt, tag="wp", name="wp")

            xpool = st.enter_context(tc.tile_pool(name="xs", bufs=2))
            apool = st.enter_context(tc.tile_pool(name="att", bufs=3))
            yrpool = st.enter_context(tc.tile_pool(name="yraw", bufs=4))
            ztpool = st.enter_context(tc.tile_pool(name="zev", bufs=2))
            # PSUM: pab 2 + ps 2x2 + po 2 = 8 banks
            pab = st.enter_context(tc.tile_pool(name="pab", bufs=2, space="PSUM"))
            pss = st.enter_context(tc.tile_pool(name="pss", bufs=2, space="PSUM"))
            pso = st.enter_context(tc.tile_pool(name="pso", bufs=2, space="PSUM"))

            for i in range(2):
                nc.vector.memset(sgs[i][:], 1.0)
                nc.vector.memset(E_bc[i][:], 0.0)
            nc.vector.memset(E_bc[0][0:1, 0:D], 1.0)
            nc.vector.memset(E_bc[0][32:33, D:P], 1.0)
            nc.vector.memset(E_bc[1][64:65, 0:D], 1.0)
            nc.vector.memset(E_bc[1][96:97, D:P], 1.0)

            def btp_norm(j, ptt, yr2p):
                # heads 2j,2j+1 live in rg tile j//2 rows 64(j%2)+{0,32}
                btp = pab.tile([P, QT], f32, tag="pab", name="btp")
                nc.tensor.matmul(btp[:], E_bc[j % 2][:], rgs[j // 2][:],
                                 start=True, stop=True)
                nc.vector.tensor_mul(
                    y_sb[j][:, ptt * QT:(ptt + 1) * QT], yr2p[j][:], btp[:])

            def cproj_chunks(ptt):
                # one thunk per output-channel block; interleaved into the
                # attention stream as PE gap-filler so the HAM clock gate
                # never sees an idle window during ACT-bound stretches
                ztile = ztpool.tile([P, 8, QT], f32, tag="zt", name="zt")

                def mk(oc):
                    def go():
                        ps = pab.tile([P, QT], f32, tag="pab", name="pab")
                        for i in range(4):
                            nc.tensor.matmul(
                                ps[:], wp_sb[:, i, oc * P:(oc + 1) * P],
                                y_sb[i][:, ptt * QT:(ptt + 1) * QT],
                                start=(i == 0), stop=(i == 3))
                        if oc >= 6:
                            # ACT is idle at tile boundaries; jumping the DVE
                            # backlog here frees the psum slot for proj(tt+1)
                            nc.scalar.activation(
                                ztile[:, oc, :], ps[:], Ident,
                                bias=bpj_sb[:, oc:oc + 1])
                        else:
                            nc.vector.tensor_scalar_add(
                                ztile[:, oc, :], ps[:], bpj_sb[:, oc:oc + 1])
                        if oc == 7:
                            nc.sync.dma_start(
                                zT[:, :, ptt * QT:(ptt + 1) * QT], ztile[:])
                    return go
                return [mk(oc) for oc in range(8)]

            prev = None
            for tt in range(NQT):
                # ---- projections for token tile tt ----
                xt = xpool.tile([P, IC, QT], mdt, tag="x", name="x")
                if tt == 0:
                    # halves so the first qk chain starts after ~1.5MB
                    h = IC // 2
                    nc.sync.dma_start(xt[:, 0:h, :],
                                      xT[:, 0:h, 0:QT])
                    nc.sync.dma_start(wqk_sb[:, 0:h, :], wqkT[:, 0:h, :])
                    nc.sync.dma_start(xt[:, h:IC, :],
                                      xT[:, h:IC, 0:QT])
                    nc.sync.dma_start(wqk_sb[:, h:IC, :], wqkT[:, h:IC, :])
                    nc.sync.dma_start(bqk_sb[:], bqk)
                    nc.sync.dma_start(wv_sb[:], wvT)
                    nc.sync.dma_start(maskf[:], maskd)
                    nc.sync.dma_start(wp_sb[:], wpT)
                    nc.sync.dma_start(bpj_sb[:], bpj)
                else:
                    nc.sync.dma_start(xt[:], xT[:, :, tt * QT:(tt + 1) * QT])
                # qk-proj: psum[out-ch 128, tok 512] accumulated over in-ch
                for oc in range(8):
                    ps = pab.tile([P, QT], f32, tag="pab", name="pab")
                    for i in range(IC):
                        nc.tensor.matmul(
                            ps[:], wqk_sb[:, i, oc * P:(oc + 1) * P],
                            xt[:, i, :], start=(i == 0), stop=(i == IC - 1))
                    nc.scalar.activation(
                        qk_sb[oc][:, tt * QT:(tt + 1) * QT], ps[:], Ident,
                        bias=bqk_sb[:, oc:oc + 1])
                # v-proj: psum[tok 128, out-ch 512] per tok block
                for tb in range(4):
                    kb = tt * 4 + tb
                    ps = pab.tile([P, NH, D], f32, tag="pab", name="pab")
                    for i in range(IC):
                        nc.tensor.matmul(
                            ps[:], xt[:, i, tb * P:(tb + 1) * P],
                            wv_sb[:, i, :], start=(i == 0), stop=(i == IC - 1))
                    nc.scalar.activation(v_sb[kb][:, :, 0:D], ps[:], Copy)
                    nc.vector.memset(v_sb[kb][:, :, D:D + 1], 1.0)

                # deferred normalize + c_proj for the previous token tile:
                # runs while this tile's attention streams, so the PE queue
                # never stalls behind the reciprocal
                chunks = []
                if prev is not None:
                    for j in range(4):
                        btp_norm(j, *prev)
                    chunks = cproj_chunks(prev[0])

                # ---- attention for query tile tt ----
                qtt = tt
                nkb = (qtt + 1) * 4
                npair = nkb // 2
                # task list: (h, pi); S+exp emitted one slot ahead of att@V
                tasks = [(h, pi) for h in range(NH) for pi in range(npair)]
                po_t = [None] * NH
                at_t = {}
                yr2 = [None] * 4

                def emit_s(h, pi):
                    p0 = (h % 2) * D
                    qt_i = h // 2
                    kt_i = 4 + h // 2
                    kbs = (2 * pi, 2 * pi + 1)
                    ns, c0s = [], []
                    for kb in kbs:
                        e = kb * P - qtt * QT
                        c0s.append(max(e, 0))
                        ns.append(QT - max(e, 0))
                    o1 = ns[0] if ns[0] + ns[1] <= QT else QT
                    width = o1 + ns[1]
                    ps = pss.tile([P, 2 * QT], f32, tag="ps", name="ps")
                    at = apool.tile([P, 2 * QT], mdt, tag="at", name="at")
                    for kb, n, c0, o in zip(kbs, ns, c0s, (0, o1)):
                        nc.tensor.matmul(
                            ps[:, o:o + n],
                            qk_sb[kt_i][p0:p0 + D, kb * P:(kb + 1) * P],
                            qk_sb[qt_i][p0:p0 + D,
                                        qtt * QT + c0:(qtt + 1) * QT],
                            start=True, stop=True)
                    nc.scalar.activation(at[:, 0:width], ps[:, 0:width],
                                         Exp, scale=0.125)
                    for kb, n, c0, o in zip(kbs, ns, c0s, (0, o1)):
                        if kb * P - qtt * QT >= 0:
                            # zero strict upper triangle; never reaches past
                            # the first 128 live columns
                            m = min(n, P)
                            nc.vector.tensor_mul(at[:, o:o + m],
                                                 at[:, o:o + m],
                                                 maskf[:, 0:m])
                    at_t[(h, pi)] = (at, ns, c0s, (0, o1))

                def emit_a(h, pi):
                    at, ns, c0s, os_ = at_t.pop((h, pi))
                    kbs = (2 * pi, 2 * pi + 1)
                    if pi == 0:
                        po_t[h] = pso.tile([D + 1, QT], f32, tag="po",
                                           name="po")
                    po = po_t[h]
                    for kb, n, c0, o in zip(kbs, ns, c0s, os_):
                        nc.tensor.matmul(
                            po[:, c0:QT], v_sb[kb][:, h, :], at[:, o:o + n],
                            start=(kb == 0), stop=(kb == nkb - 1))
                    if pi == npair - 1:
                        # evict numerator into the paired-head tile and the
                        # denominator row into its sg slot; frees the bank
                        j = h // 2
                        if yr2[j] is None:
                            yr2[j] = yrpool.tile([P, QT], f32, tag="yr",
                                                 name="yr")
                        r0 = D * (h % 2)
                        nc.vector.tensor_copy(yr2[j][r0:r0 + D, :], po[0:D, :])
                        nc.vector.tensor_copy(
                            sgs[h // 4][32 * (h % 4):32 * (h % 4) + 1, :],
                            po[D:D + 1, :])
                        po_t[h] = None

                last = NQT - 1
                stride = max(2, (3 * len(tasks) // 4) // max(len(chunks), 1))
                ci = 0
                for s, t in enumerate(tasks):
                    emit_s(*t)
                    if s >= 1:
                        emit_a(*tasks[s - 1])
                        if ci < len(chunks) and s % stride == stride - 1:
                            chunks[ci]()
                            ci += 1
                        if tt == last and tasks[s - 1] == (3, npair - 1):
                            # final tile: normalize heads 0-3 mid-stream so
                            # only half the chain lands on the kernel tail
                            nc.vector.reciprocal(rgs[0][:], sgs[0][:])
                            btp_norm(0, tt, yr2)
                            btp_norm(1, tt, yr2)
                emit_a(*tasks[-1])
                while ci < len(chunks):
                    chunks[ci]()
                    ci += 1
                if tt == last:
                    nc.vector.reciprocal(rgs[1][:], sgs[1][:])
                    btp_norm(2, tt, yr2)
                    btp_norm(3, tt, yr2)
                    for go in cproj_chunks(tt):
                        go()
                else:
                    for g in range(2):
                        nc.vector.reciprocal(rgs[g][:], sgs[g][:])
                    prev = (tt, yr2)
    nc.compile()
    return nc


def get_nc():
    if "nc" not in _nc_cache:
        _nc_cache["nc"] = _build_nc()
    return _nc_cache["nc"]


def _mm_np_dtype():
    if MM_DT == "bfloat16":
        import ml_dtypes
        return np.dtype(ml_dtypes.bfloat16)
    return np.dtype(np.float32)


def _blk(a, nb):
    """[nb*128, F] -> [128, nb, F] with out[p, i, f] = a[i*128+p, f]."""
    return np.ascontiguousarray(
        a.reshape(nb, P, -1).transpose(1, 0, 2))


def make_in_maps(x, Wqkv, bqkv, Wproj, bproj):
    x = np.asarray(x, np.float32)
    Wqkv = np.asarray(Wqkv, np.float32)
    bqkv = np.asarray(bqkv, np.float32)
    Wproj = np.asarray(Wproj, np.float32)
    bproj = np.asarray(bproj, np.float32)
    Wq, Wk, Wv = Wqkv[0:C], Wqkv[C:2 * C], Wqkv[2 * C:3 * C]
    bq, bk, bv = bqkv[0:C], bqkv[C:2 * C], bqkv[2 * C:3 * C]
    mdt = _mm_np_dtype()
    mask = (np.arange(QT)[None, :] >= np.arange(P)[:, None]).astype(mdt)
    in_maps = []
    for b in range(B):
        xTb = _blk(x[b].T.astype(mdt), IC)
        for s in range(2):
            cols = slice(s * LC, (s + 1) * LC)
            wqkT = _blk(np.concatenate(
                [Wq[cols], Wk[cols]], 0).T.astype(mdt), IC)
            bqk_ = np.concatenate([bq[cols], bk[cols]])
            wvT_ = _blk(Wv[cols].T.astype(mdt), IC)
            wpT_ = _blk(Wproj[:, cols].T.astype(mdt), 4)
            bp_eff = bv[cols] @ Wproj[:, cols].T
            if s == 0:
                bp_eff = bp_eff + bproj
            in_maps.append({
                "xT": xTb,
                "wqkT": wqkT,
                "bqk": np.ascontiguousarray(bqk_.reshape(8, P).T),
                "wvT": wvT_,
                "wpT": wpT_,
                "bpj": np.ascontiguousarray(
                    bp_eff.astype(np.float32).reshape(8, P).T),
                "mask": mask,
            })
    return in_maps


def gather_out(results):
    out = np.empty((B, T, C), np.float32)
    for b in range(B):
        zt = results[2 * b]["zT"] + results[2 * b + 1]["zT"]
        # zt[p, oc, t] -> z[t, oc*128+p]
        out[b] = zt.transpose(1, 0, 2).reshape(C, T).T
    return out


def kernel(x, Wqkv, bqkv, Wproj, bproj):
    from concourse.bass_utils import run_bass_kernel_spmd

    in_maps = make_in_maps(x, Wqkv, bqkv, Wproj, bproj)
    try:
        res = run_bass_kernel_spmd(get_nc(), in_maps, core_ids=list(range(8)))
    except Exception:
        # transient device faults have been observed once; retry a single time
        res = run_bass_kernel_spmd(get_nc(), in_maps, core_ids=list(range(8)))
    return gather_out(res.results)


# revision 13
# speedup vs baseline: 1.6326x; 1.0277x over previous
"""Causal self-attention (B=4, T=2048, C=1024, H=16) on 8 trn2 NeuronCores.

Sharding: core = (batch b, head-half s).  Each core computes q/k/v
projections for its 8 heads (weights pre-sliced/transposed on host),
causal flash-style attention with transposed score tiles, and a partial
(row-sharded) c_proj.  Host gather sums the two partials per batch.

v4: bf16 matmul inputs; proj/attention merged per token tile; attention
software-pipelined (S-pair emitted one slot ahead of its att@V consumer
so the PE never waits on exp); normalize+cproj of tile tt deferred past
proj(tt+1) so the slow DVE reciprocal never heads the PE queue; host
packs weights/x/z into [128, blk, free] so each tensor is one big DMA.

Device data layout (bf16 unless noted):
  xT    [128, 8, 2048]  x[b].T blocked     xT[p,i,t] = x[b][t, i*128+p]
  wqkT  [128, 8, 1024]  [Wq_l | Wk_l].T blocked
  bqk   [128, 8] f32    q/k bias, per out-ch block
  wvT   [128, 8, 512]   Wv_local.T blocked
  wpT   [128, 4, 1024]  Wproj[:, local].T blocked
  bpj   [128, 8] f32    bproj + bv@WprojT (folded), half per core
  mask  [128, 512]      causal keep-mask m[p,c] = (c >= p)
  zT    [128, 8, 2048] f32  partial out, z[t, oc*128+p] = zT[p,oc,t]
"""

import os
import sys

sys.path.insert(0, "/opt/trn_rl_repo")

import numpy as np

B, T, C, H = 4, 2048, 1024, 16
D = 64          # head dim
NH = 8          # heads per core
LC = NH * D     # local channels = 512
P = 128
QT = 512        # query tile (also matmul moving free dim)
NQT = T // QT   # 4
NKB = T // P    # 16 key blocks
IC = C // P     # 8 input-channel blocks

# matmul input dtype: bfloat16 = full-rate, float32r fallback (exact-ish)
MM_DT = os.environ.get("BASS_ATTN_MM_DT", "bfloat16")

_nc_cache = {}


def _build_nc():
    from contextlib import ExitStack

    import concourse.bass as bass  # noqa: F401
    import concourse.mybir as mybir
    from concourse import bacc, tile

    f32 = mybir.dt.float32
    mdt = getattr(mybir.dt, MM_DT)
    Exp = mybir.ActivationFunctionType.Exp
    Ident = mybir.ActivationFunctionType.Identity
    Copy = mybir.ActivationFunctionType.Copy

    nc = bacc.Bacc("TRN2", target_bir_lowering=False, debug=False, num_devices=8)
    xT = nc.dram_tensor("xT", [P, IC, T], mdt, kind="ExternalInput").ap()
    wqkT = nc.dram_tensor("wqkT", [P, IC, 2 * LC], mdt, kind="ExternalInput").ap()
    bqk = nc.dram_tensor("bqk", [P, 8], f32, kind="ExternalInput").ap()
    wvT = nc.dram_tensor("wvT", [P, IC, LC], mdt, kind="ExternalInput").ap()
    wpT = nc.dram_tensor("wpT", [P, 4, C], mdt, kind="ExternalInput").ap()
    bpj = nc.dram_tensor("bpj", [P, 8], f32, kind="ExternalInput").ap()
    maskd = nc.dram_tensor("mask", [P, QT], mdt, kind="ExternalInput").ap()
    zT = nc.dram_tensor("zT", [P, 8, T], f32, kind="ExternalOutput").ap()

    with tile.TileContext(nc) as tc:
        with ExitStack() as st:
            st.enter_context(nc.allow_low_precision(
                reason="bf16 throughput; accuracy checked vs reference"))
            persist = st.enter_context(tc.tile_pool(name="persist", bufs=1))
            # qk_sb: out-ch blocks 0-3 = q, 4-7 = k; [out-ch 128, tok 2048]
            qk_sb = [persist.tile([P, T], mdt, tag=f"qk{i}", name=f"qk{i}")
                     for i in range(8)]
            # v_sb[kb]: [tok 128, head 8, d 64 + ones col]
            v_sb = [persist.tile([P, NH, D + 1], mdt, tag=f"v{i}", name=f"v{i}")
                    for i in range(NKB)]
            # y_sb: attention out, [local-ch 128, tok 2048] x 4 blocks
            y_sb = [persist.tile([P, T], mdt, tag=f"y{i}", name=f"y{i}")
                    for i in range(4)]
            bqk_sb = persist.tile([P, 8], f32, tag="bqk", name="bqk")
            bpj_sb = persist.tile([P, 8], f32, tag="bpj", name="bpj")
            maskf = persist.tile([P, QT], mdt, tag="maskf", name="maskf")
            # softmax-sum rows: 4 heads per tile at 32-aligned partitions;
            # memset once so dead rows never produce inf/NaN via reciprocal
            sgs = [persist.tile([P, QT], f32, tag=f"sg{i}", name=f"sg{i}")
                   for i in range(2)]
            rgs = [persist.tile([P, QT], mdt, tag=f"rg{i}", name=f"rg{i}")
                   for i in range(2)]
            rgf = [persist.tile([P, QT], f32, tag=f"rf{i}", name=f"rf{i}")
                   for i in range(2)]
            # E matrices: block row-broadcast for paired-head normalization.
            # btp[p,c] = sum_k E[k,p]*rg[k,c]; E_lo rows {0,32}, E_hi {64,96}
            E_bc = [persist.tile([P, P], mdt, tag=f"E{i}", name=f"E{i}")
                    for i in range(2)]
            # weights, host-packed [128, blk, free]
            wqk_sb = persist.tile([P, IC, 2 * LC], mdt, tag="wqk", name="wqk")
            wv_sb = persist.tile([P, IC, LC], mdt, tag="wv", name="wv")
            wp_sb = persist.tile([P, 4, C], mdt, tag="wp", name="wp")

            xpool = st.enter_context(tc.tile_pool(name="xs", bufs=2))
            apool = st.enter_context(tc.tile_pool(name="att", bufs=3))
            yrpool = st.enter_context(tc.tile_pool(name="yraw", bufs=4))
            ztpool = st.enter_context(tc.tile_pool(name="zev", bufs=2))
            # PSUM: pab 2 + ps 2x2 + po 2 = 8 banks
            pab = st.enter_context(tc.tile_pool(name="pab", bufs=2, space="PSUM"))
            pss = st.enter_context(tc.tile_pool(name="pss", bufs=2, space="PSUM"))
            pso = st.enter_context(tc.tile_pool(name="pso", bufs=2, space="PSUM"))

            for i in range(2):
                nc.vector.memset(sgs[i][:], 1.0)
                nc.vector.memset(E_bc[i][:], 0.0)
            nc.vector.memset(E_bc[0][0:1, 0:D], 1.0)
            nc.vector.memset(E_bc[0][32:33, D:P], 1.0)
            nc.vector.memset(E_bc[1][64:65, 0:D], 1.0)
            nc.vector.memset(E_bc[1][96:97, D:P], 1.0)

            def btp_norm(j, ptt, yr2p):
                # heads 2j,2j+1 live in rg tile j//2 rows 64(j%2)+{0,32}
                btp = pab.tile([P, QT], f32, tag="pab", name="btp")
                nc.tensor.matmul(btp[:], E_bc[j % 2][:], rgs[j // 2][:],
                                 start=True, stop=True)
                nc.vector.tensor_mul(
                    y_sb[j][:, ptt * QT:(ptt + 1) * QT], yr2p[j][:], btp[:])

            def cproj_chunks(ptt):
                # one thunk per output-channel block; interleaved into the
                # attention stream as PE gap-filler so the HAM clock gate
                # never sees an idle window during ACT-bound stretches
                ztile = ztpool.tile([P, 8, QT], f32, tag="zt", name="zt")

                def mk(oc):
                    def go():
                        ps = pab.tile([P, QT], f32, tag="pab", name="pab")
                        for i in range(4):
                            nc.tensor.matmul(
                                ps[:], wp_sb[:, i, oc * P:(oc + 1) * P],
                                y_sb[i][:, ptt * QT:(ptt + 1) * QT],
                                start=(i == 0), stop=(i == 3))
                        if oc >= 6:
                            # ACT is idle at tile boundaries; jumping the DVE
                            # backlog here frees the psum slot for proj(tt+1)
                            nc.scalar.activation(
                                ztile[:, oc, :], ps[:], Ident,
                                bias=bpj_sb[:, oc:oc + 1])
                        else:
                            nc.vector.tensor_scalar_add(
                                ztile[:, oc, :], ps[:], bpj_sb[:, oc:oc + 1])
                        if oc == 7:
                            nc.sync.dma_start(
                                zT[:, :, ptt * QT:(ptt + 1) * QT], ztile[:])
                    return go
                return [mk(oc) for oc in range(8)]

            prev = None
            for tt in range(NQT):
                # ---- projections for token tile tt ----
                xt = xpool.tile([P, IC, QT], mdt, tag="x", name="x")
                if tt == 0:
                    # small first chunk so the first qk chain starts early
                    nc.sync.dma_start(xt[:, 0:2, :], xT[:, 0:2, 0:QT])
                    nc.sync.dma_start(wqk_sb[:, 0:2, :], wqkT[:, 0:2, :])
                    nc.sync.dma_start(xt[:, 2:IC, :], xT[:, 2:IC, 0:QT])
                    nc.sync.dma_start(wqk_sb[:, 2:IC, :], wqkT[:, 2:IC, :])
                    nc.sync.dma_start(bqk_sb[:], bqk)
                    nc.sync.dma_start(wv_sb[:], wvT)
                    nc.sync.dma_start(maskf[:], maskd)
                    nc.sync.dma_start(wp_sb[:], wpT)
                    nc.sync.dma_start(bpj_sb[:], bpj)
                else:
                    nc.sync.dma_start(xt[:], xT[:, :, tt * QT:(tt + 1) * QT])
                # qk-proj: psum[out-ch 128, tok 512] accumulated over in-ch
                for oc in range(8):
                    ps = pab.tile([P, QT], f32, tag="pab", name="pab")
                    for i in range(IC):
                        nc.tensor.matmul(
                            ps[:], wqk_sb[:, i, oc * P:(oc + 1) * P],
                            xt[:, i, :], start=(i == 0), stop=(i == IC - 1))
                    nc.scalar.activation(
                        qk_sb[oc][:, tt * QT:(tt + 1) * QT], ps[:], Ident,
                        bias=bqk_sb[:, oc:oc + 1])
                # v-proj: psum[tok 128, out-ch 512] per tok block
                for tb in range(4):
                    kb = tt * 4 + tb
                    ps = pab.tile([P, NH, D], f32, tag="pab", name="pab")
                    for i in range(IC):
                        nc.tensor.matmul(
                            ps[:], xt[:, i, tb * P:(tb + 1) * P],
                            wv_sb[:, i, :], start=(i == 0), stop=(i == IC - 1))
                    nc.scalar.activation(v_sb[kb][:, :, 0:D], ps[:], Copy)
                    nc.vector.memset(v_sb[kb][:, :, D:D + 1], 1.0)

                # deferred normalize + c_proj for the previous token tile:
                # runs while this tile's attention streams, so the PE queue
                # never stalls behind the reciprocal
                fillers = []
                if prev is not None:
                    pvt, yr2p = prev
                    for j in range(4):
                        fillers.append(
                            lambda j=j: btp_norm(j, pvt, yr2p))
                    fillers += cproj_chunks(pvt)

                # ---- attention for query tile tt ----
                qtt = tt
                nkb = (qtt + 1) * 4
                npair = nkb // 2
                # task list: (h, pi); S+exp emitted one slot ahead of att@V
                tasks = [(h, pi) for h in range(NH) for pi in range(npair)]
                po_t = [None] * NH
                at_t = {}
                yr2 = [None] * 4

                def emit_s(h, pi):
                    p0 = (h % 2) * D
                    qt_i = h // 2
                    kt_i = 4 + h // 2
                    kbs = (2 * pi, 2 * pi + 1)
                    ns, c0s = [], []
                    for kb in kbs:
                        e = kb * P - qtt * QT
                        c0s.append(max(e, 0))
                        ns.append(QT - max(e, 0))
                    o1 = ns[0] if ns[0] + ns[1] <= QT else QT
                    width = o1 + ns[1]
                    ps = pss.tile([P, 2 * QT], f32, tag="ps", name="ps")
                    at = apool.tile([P, 2 * QT], mdt, tag="at", name="at")
                    for kb, n, c0, o in zip(kbs, ns, c0s, (0, o1)):
                        nc.tensor.matmul(
                            ps[:, o:o + n],
                            qk_sb[kt_i][p0:p0 + D, kb * P:(kb + 1) * P],
                            qk_sb[qt_i][p0:p0 + D,
                                        qtt * QT + c0:(qtt + 1) * QT],
                            start=True, stop=True)
                    nc.scalar.activation(at[:, 0:width], ps[:, 0:width],
                                         Exp, scale=0.125)
                    for kb, n, c0, o in zip(kbs, ns, c0s, (0, o1)):
                        if kb * P - qtt * QT >= 0:
                            # zero strict upper triangle; never reaches past
                            # the first 128 live columns
                            m = min(n, P)
                            nc.vector.tensor_mul(at[:, o:o + m],
                                                 at[:, o:o + m],
                                                 maskf[:, 0:m])
                    at_t[(h, pi)] = (at, ns, c0s, (0, o1))

                def emit_a(h, pi):
                    at, ns, c0s, os_ = at_t.pop((h, pi))
                    kbs = (2 * pi, 2 * pi + 1)
                    if pi == 0:
                        po_t[h] = pso.tile([D + 1, QT], f32, tag="po",
                                           name="po")
                    po = po_t[h]
                    for kb, n, c0, o in zip(kbs, ns, c0s, os_):
                        nc.tensor.matmul(
                            po[:, c0:QT], v_sb[kb][:, h, :], at[:, o:o + n],
                            start=(kb == 0), stop=(kb == nkb - 1))
                    if pi == npair - 1:
                        # evict numerator into the paired-head tile and the
                        # denominator row into its sg slot; frees the bank
                        j = h // 2
                        if yr2[j] is None:
                            yr2[j] = yrpool.tile([P, QT], f32, tag="yr",
                                                 name="yr")
                        r0 = D * (h % 2)
                        nc.vector.tensor_copy(yr2[j][r0:r0 + D, :], po[0:D, :])
                        nc.vector.tensor_copy(
                            sgs[h // 4][32 * (h % 4):32 * (h % 4) + 1, :],
                            po[D:D + 1, :])
                        po_t[h] = None

                def recip(g):
                    # ~5x faster than the iterative DVE reciprocal; sums are
                    # positive normals so the approx edge cases can't occur
                    nc.vector.reciprocal_approx_fast(rgf[g][:], sgs[g][:])
                    nc.vector.tensor_copy(rgs[g][:], rgf[g][:])

                last = NQT - 1
                zpart = None
                ztile_l = None
                if tt == last:
                    zpart = ztpool.tile([P, 8, QT], f32, tag="zp", name="zp")
                    ztile_l = ztpool.tile([P, 8, QT], f32, tag="zt", name="zt")

                def phase_a(oc):
                    # first half of the final c_proj: runs during the tail
                    # reciprocal so the PE never idles at kernel end
                    ps = pab.tile([P, QT], f32, tag="pab", name="pab")
                    for i in range(2):
                        nc.tensor.matmul(
                            ps[:], wp_sb[:, i, oc * P:(oc + 1) * P],
                            y_sb[i][:, tt * QT:(tt + 1) * QT],
                            start=(i == 0), stop=(i == 1))
                    nc.vector.tensor_scalar_add(zpart[:, oc, :], ps[:],
                                                bpj_sb[:, oc:oc + 1])

                for s, t in enumerate(tasks):
                    emit_s(*t)
                    if s >= 1:
                        emit_a(*tasks[s - 1])
                        if tt == last and tasks[s - 1] == (3, npair - 1):
                            # final tile: normalize heads 0-3 mid-stream so
                            # only half the chain lands on the kernel tail
                            recip(0)
                            btp_norm(0, tt, yr2)
                            btp_norm(1, tt, yr2)
                            fillers += [
                                (lambda oc=oc: phase_a(oc)) for oc in range(8)]
                    if s >= 2 and fillers:
                        fillers.pop(0)()
                emit_a(*tasks[-1])
                for go in fillers:
                    go()
                if tt == last:
                    recip(1)
                    btp_norm(2, tt, yr2)
                    btp_norm(3, tt, yr2)
                    for oc in range(8):
                        ps = pab.tile([P, QT], f32, tag="pab", name="pab")
                        for i in range(2, 4):
                            nc.tensor.matmul(
                                ps[:], wp_sb[:, i, oc * P:(oc + 1) * P],
                                y_sb[i][:, tt * QT:(tt + 1) * QT],
                                start=(i == 2), stop=(i == 3))
                        nc.vector.tensor_add(ztile_l[:, oc, :], ps[:],
                                             zpart[:, oc, :])
                        if oc == 7:
                            nc.sync.dma_start(
                                zT[:, :, tt * QT:(tt + 1) * QT], ztile_l[:])
                else:
                    recip(0)
                    recip(1)
                    prev = (tt, yr2)
    nc.compile()
    return nc


def get_nc():
    if "nc" not in _nc_cache:
        _nc_cache["nc"] = _build_nc()
    return _nc_cache["nc"]


def _mm_np_dtype():
    if MM_DT == "bfloat16":
        import ml_dtypes
        return np.dtype(ml_dtypes.bfloat16)
    return np.dtype(np.float32)


def _blk(a, nb):
    """[nb*128, F] -> [128, nb, F] with out[p, i, f] = a[i*128+p, f]."""
    return np.ascontiguousarray(
        a.reshape(nb, P, -1).transpose(1, 0, 2))


def make_in_maps(x, Wqkv, bqkv, Wproj, bproj):
    x = np.asarray(x, np.float32)
    Wqkv = np.asarray(Wqkv, np.float32)
    bqkv = np.asarray(bqkv, np.float32)
    Wproj = np.asarray(Wproj, np.float32)
    bproj = np.asarray(bproj, np.float32)
    Wq, Wk, Wv = Wqkv[0:C], Wqkv[C:2 * C], Wqkv[2 * C:3 * C]
    bq, bk, bv = bqkv[0:C], bqkv[C:2 * C], bqkv[2 * C:3 * C]
    mdt = _mm_np_dtype()
    mask = (np.arange(QT)[None, :] >= np.arange(P)[:, None]).astype(mdt)
    in_maps = []
    for b in range(B):
        xTb = _blk(x[b].T.astype(mdt), IC)
        for s in range(2):
            cols = slice(s * LC, (s + 1) * LC)
            wqkT = _blk(np.concatenate(
                [Wq[cols], Wk[cols]], 0).T.astype(mdt), IC)
            bqk_ = np.concatenate([bq[cols], bk[cols]])
            wvT_ = _blk(Wv[cols].T.astype(mdt), IC)
            wpT_ = _blk(Wproj[:, cols].T.astype(mdt), 4)
            bp_eff = bv[cols] @ Wproj[:, cols].T
            if s == 0:
                bp_eff = bp_eff + bproj
            in_maps.append({
                "xT": xTb,
                "wqkT": wqkT,
                "bqk": np.ascontiguousarray(bqk_.reshape(8, P).T),
                "wvT": wvT_,
                "wpT": wpT_,
                "bpj": np.ascontiguousarray(
                    bp_eff.astype(np.float32).reshape(8, P).T),
                "mask": mask,
            })
    return in_maps


def gather_out(results):
    out = np.empty((B, T, C), np.float32)
    for b in range(B):
        zt = results[2 * b]["zT"] + results[2 * b + 1]["zT"]
        # zt[p, oc, t] -> z[t, oc*128+p]
        out[b] = zt.transpose(1, 0, 2).reshape(C, T).T
    return out


def kernel(x, Wqkv, bqkv, Wproj, bproj):
    from concourse.bass_utils import run_bass_kernel_spmd

    in_maps = make_in_maps(x, Wqkv, bqkv, Wproj, bproj)
    try:
        res = run_bass_kernel_spmd(get_nc(), in_maps, core_ids=list(range(8)))
    except Exception:
        # transient device faults have been observed once; retry a single time
        res = run_bass_kernel_spmd(get_nc(), in_maps, core_ids=list(range(8)))
    return gather_out(res.results)


# revision 15
# speedup vs baseline: 1.6465x; 1.0085x over previous
"""Causal self-attention (B=4, T=2048, C=1024, H=16) on 8 trn2 NeuronCores.

Sharding: core = (batch b, head-half s).  Each core computes q/k/v
projections for its 8 heads (weights pre-sliced/transposed on host),
causal flash-style attention with transposed score tiles, and a partial
(row-sharded) c_proj.  Host gather sums the two partials per batch.

v4: bf16 matmul inputs; proj/attention merged per token tile; attention
software-pipelined (S-pair emitted one slot ahead of its att@V consumer
so the PE never waits on exp); normalize+cproj of tile tt deferred past
proj(tt+1) so the slow DVE reciprocal never heads the PE queue; host
packs weights/x/z into [128, blk, free] so each tensor is one big DMA.

Device data layout (bf16 unless noted):
  xT    [128, 8, 2048]  x[b].T blocked     xT[p,i,t] = x[b][t, i*128+p]
  wqkT  [128, 8, 1024]  [Wq_l | Wk_l].T blocked
  bqk   [128, 8] f32    q/k bias, per out-ch block
  wvT   [128, 8, 512]   Wv_local.T blocked
  wpT   [128, 4, 1024]  Wproj[:, local].T blocked
  bpj   [128, 8] f32    bproj + bv@WprojT (folded), half per core
  mask  [128, 512]      causal keep-mask m[p,c] = (c >= p)
  zT    [128, 8, 2048] f32  partial out, z[t, oc*128+p] = zT[p,oc,t]
"""

import os
import sys

sys.path.insert(0, "/opt/trn_rl_repo")

import numpy as np

B, T, C, H = 4, 2048, 1024, 16
D = 64          # head dim
NH = 8          # heads per core
LC = NH * D     # local channels = 512
P = 128
QT = 512        # query tile (also matmul moving free dim)
NQT = T // QT   # 4
NKB = T // P    # 16 key blocks
IC = C // P     # 8 input-channel blocks

# matmul input dtype: bfloat16 = full-rate, float32r fallback (exact-ish)
MM_DT = os.environ.get("BASS_ATTN_MM_DT", "bfloat16")

_nc_cache = {}


def _build_nc():
    from contextlib import ExitStack

    import concourse.bass as bass  # noqa: F401
    import concourse.mybir as mybir
    from concourse import bacc, tile

    f32 = mybir.dt.float32
    mdt = getattr(mybir.dt, MM_DT)
    Exp = mybir.ActivationFunctionType.Exp
    Ident = mybir.ActivationFunctionType.Identity
    Copy = mybir.ActivationFunctionType.Copy

    nc = bacc.Bacc("TRN2", target_bir_lowering=False, debug=False, num_devices=8)
    xT = nc.dram_tensor("xT", [P, IC, T], mdt, kind="ExternalInput").ap()
    wqkT = nc.dram_tensor("wqkT", [P, IC, 2 * LC], mdt, kind="ExternalInput").ap()
    bqk = nc.dram_tensor("bqk", [P, 8], f32, kind="ExternalInput").ap()
    wvT = nc.dram_tensor("wvT", [P, IC, LC], mdt, kind="ExternalInput").ap()
    wpT = nc.dram_tensor("wpT", [P, 4, C], mdt, kind="ExternalInput").ap()
    bpj = nc.dram_tensor("bpj", [P, 8], f32, kind="ExternalInput").ap()
    maskd = nc.dram_tensor("mask", [P, QT], mdt, kind="ExternalInput").ap()
    zT = nc.dram_tensor("zT", [P, 8, T], f32, kind="ExternalOutput").ap()

    with tile.TileContext(nc) as tc:
        with ExitStack() as st:
            st.enter_context(nc.allow_low_precision(
                reason="bf16 throughput; accuracy checked vs reference"))
            persist = st.enter_context(tc.tile_pool(name="persist", bufs=1))
            # qk_sb: out-ch blocks 0-3 = q, 4-7 = k; [out-ch 128, tok 2048]
            qk_sb = [persist.tile([P, T], mdt, tag=f"qk{i}", name=f"qk{i}")
                     for i in range(8)]
            # v_sb[kb]: [tok 128, head 8, d 64 + ones col]
            v_sb = [persist.tile([P, NH, D + 1], mdt, tag=f"v{i}", name=f"v{i}")
                    for i in range(NKB)]
            # y_sb: attention out, [local-ch 128, tok 2048] x 4 blocks
            y_sb = [persist.tile([P, T], mdt, tag=f"y{i}", name=f"y{i}")
                    for i in range(4)]
            bqk_sb = persist.tile([P, 8], f32, tag="bqk", name="bqk")
            bpj_sb = persist.tile([P, 8], f32, tag="bpj", name="bpj")
            maskf = persist.tile([P, QT], mdt, tag="maskf", name="maskf")
            # softmax-sum rows: 4 heads per tile at 32-aligned partitions;
            # memset once so dead rows never produce inf/NaN via reciprocal
            sgs = [persist.tile([P, QT], f32, tag=f"sg{i}", name=f"sg{i}")
                   for i in range(2)]
            rgs = [persist.tile([P, QT], mdt, tag=f"rg{i}", name=f"rg{i}")
                   for i in range(2)]
            rgf = [persist.tile([P, QT], f32, tag=f"rf{i}", name=f"rf{i}")
                   for i in range(2)]
            # E matrices: block row-broadcast for paired-head normalization.
            # btp[p,c] = sum_k E[k,p]*rg[k,c]; E_lo rows {0,32}, E_hi {64,96}
            E_bc = [persist.tile([P, P], mdt, tag=f"E{i}", name=f"E{i}")
                    for i in range(2)]
            # weights, host-packed [128, blk, free]
            wqk_sb = persist.tile([P, IC, 2 * LC], mdt, tag="wqk", name="wqk")
            wv_sb = persist.tile([P, IC, LC], mdt, tag="wv", name="wv")
            wp_sb = persist.tile([P, 4, C], mdt, tag="wp", name="wp")

            xpool = st.enter_context(tc.tile_pool(name="xs", bufs=2))
            apool = st.enter_context(tc.tile_pool(name="att", bufs=3))
            yrpool = st.enter_context(tc.tile_pool(name="yraw", bufs=4))
            ztpool = st.enter_context(tc.tile_pool(name="zev", bufs=2))
            # PSUM: pab 2 + ps 2x2 + po 2 = 8 banks
            pab = st.enter_context(tc.tile_pool(name="pab", bufs=2, space="PSUM"))
            pss = st.enter_context(tc.tile_pool(name="pss", bufs=2, space="PSUM"))
            pso = st.enter_context(tc.tile_pool(name="pso", bufs=2, space="PSUM"))

            for i in range(2):
                nc.vector.memset(sgs[i][:], 1.0)
                nc.vector.memset(E_bc[i][:], 0.0)
            nc.vector.memset(E_bc[0][0:1, 0:D], 1.0)
            nc.vector.memset(E_bc[0][32:33, D:P], 1.0)
            nc.vector.memset(E_bc[1][64:65, 0:D], 1.0)
            nc.vector.memset(E_bc[1][96:97, D:P], 1.0)

            def btp_norm(j, ptt, yr2p):
                # heads 2j,2j+1 live in rg tile j//2 rows 64(j%2)+{0,32}
                btp = pab.tile([P, QT], f32, tag="pab", name="btp")
                nc.tensor.matmul(btp[:], E_bc[j % 2][:], rgs[j // 2][:],
                                 start=True, stop=True)
                nc.vector.tensor_mul(
                    y_sb[j][:, ptt * QT:(ptt + 1) * QT], yr2p[j][:], btp[:])

            def cproj_chunks(ptt):
                # one thunk per output-channel block; interleaved into the
                # attention stream as PE gap-filler so the HAM clock gate
                # never sees an idle window during ACT-bound stretches
                ztile = ztpool.tile([P, 8, QT], f32, tag="zt", name="zt")

                def mk(oc):
                    def go():
                        ps = pab.tile([P, QT], f32, tag="pab", name="pab")
                        for i in range(4):
                            nc.tensor.matmul(
                                ps[:], wp_sb[:, i, oc * P:(oc + 1) * P],
                                y_sb[i][:, ptt * QT:(ptt + 1) * QT],
                                start=(i == 0), stop=(i == 3))
                        if oc >= 6:
                            # ACT is idle at tile boundaries; jumping the DVE
                            # backlog here frees the psum slot for proj(tt+1)
                            nc.scalar.activation(
                                ztile[:, oc, :], ps[:], Ident,
                                bias=bpj_sb[:, oc:oc + 1])
                        else:
                            nc.vector.tensor_scalar_add(
                                ztile[:, oc, :], ps[:], bpj_sb[:, oc:oc + 1])
                        if oc == 7:
                            nc.sync.dma_start(
                                zT[:, :, ptt * QT:(ptt + 1) * QT], ztile[:])
                    return go
                return [mk(oc) for oc in range(8)]

            prev = None
            for tt in range(NQT):
                # ---- projections for token tile tt ----
                xt = xpool.tile([P, IC, QT], mdt, tag="x", name="x")
                if tt == 0:
                    # interleaved 2-block chunks pace the first qk chain
                    for c0 in range(0, IC, 2):
                        nc.sync.dma_start(xt[:, c0:c0 + 2, :],
                                          xT[:, c0:c0 + 2, 0:QT])
                        nc.sync.dma_start(wqk_sb[:, c0:c0 + 2, :],
                                          wqkT[:, c0:c0 + 2, :])
                    nc.sync.dma_start(bqk_sb[:], bqk)
                    nc.sync.dma_start(wv_sb[:], wvT)
                    nc.sync.dma_start(maskf[:], maskd)
                    nc.sync.dma_start(wp_sb[:], wpT)
                    nc.sync.dma_start(bpj_sb[:], bpj)
                else:
                    nc.sync.dma_start(xt[:], xT[:, :, tt * QT:(tt + 1) * QT])
                # qk-proj: psum[out-ch 128, tok 512] accumulated over in-ch
                for oc in range(8):
                    ps = pab.tile([P, QT], f32, tag="pab", name="pab")
                    for i in range(IC):
                        nc.tensor.matmul(
                            ps[:], wqk_sb[:, i, oc * P:(oc + 1) * P],
                            xt[:, i, :], start=(i == 0), stop=(i == IC - 1))
                    nc.scalar.activation(
                        qk_sb[oc][:, tt * QT:(tt + 1) * QT], ps[:], Ident,
                        bias=bqk_sb[:, oc:oc + 1])
                # v-proj: psum[tok 128, out-ch 512] per tok block
                for tb in range(4):
                    kb = tt * 4 + tb
                    ps = pab.tile([P, NH, D], f32, tag="pab", name="pab")
                    for i in range(IC):
                        nc.tensor.matmul(
                            ps[:], xt[:, i, tb * P:(tb + 1) * P],
                            wv_sb[:, i, :], start=(i == 0), stop=(i == IC - 1))
                    nc.scalar.activation(v_sb[kb][:, :, 0:D], ps[:], Copy)
                    nc.vector.memset(v_sb[kb][:, :, D:D + 1], 1.0)

                # deferred normalize + c_proj for the previous token tile:
                # runs while this tile's attention streams, so the PE queue
                # never stalls behind the reciprocal
                fillers = []
                if prev is not None:
                    pvt, yr2p = prev
                    for j in range(4):
                        fillers.append(
                            lambda j=j: btp_norm(j, pvt, yr2p))
                    fillers += cproj_chunks(pvt)

                # ---- attention for query tile tt ----
                qtt = tt
                nkb = (qtt + 1) * 4
                npair = nkb // 2
                # task list: (h, pi); S+exp emitted one slot ahead of att@V
                tasks = [(h, pi) for h in range(NH) for pi in range(npair)]
                po_t = [None] * NH
                at_t = {}
                yr2 = [None] * 4

                def emit_s(h, pi):
                    p0 = (h % 2) * D
                    qt_i = h // 2
                    kt_i = 4 + h // 2
                    kbs = (2 * pi, 2 * pi + 1)
                    ns, c0s = [], []
                    for kb in kbs:
                        e = kb * P - qtt * QT
                        c0s.append(max(e, 0))
                        ns.append(QT - max(e, 0))
                    o1 = ns[0] if ns[0] + ns[1] <= QT else QT
                    width = o1 + ns[1]
                    ps = pss.tile([P, 2 * QT], f32, tag="ps", name="ps")
                    at = apool.tile([P, 2 * QT], mdt, tag="at", name="at")
                    for kb, n, c0, o in zip(kbs, ns, c0s, (0, o1)):
                        nc.tensor.matmul(
                            ps[:, o:o + n],
                            qk_sb[kt_i][p0:p0 + D, kb * P:(kb + 1) * P],
                            qk_sb[qt_i][p0:p0 + D,
                                        qtt * QT + c0:(qtt + 1) * QT],
                            start=True, stop=True)
                    nc.scalar.activation(at[:, 0:width], ps[:, 0:width],
                                         Exp, scale=0.125)
                    for kb, n, c0, o in zip(kbs, ns, c0s, (0, o1)):
                        if kb * P - qtt * QT >= 0:
                            # zero strict upper triangle; never reaches past
                            # the first 128 live columns
                            m = min(n, P)
                            nc.vector.tensor_mul(at[:, o:o + m],
                                                 at[:, o:o + m],
                                                 maskf[:, 0:m])
                    at_t[(h, pi)] = (at, ns, c0s, (0, o1))

                def emit_a(h, pi):
                    at, ns, c0s, os_ = at_t.pop((h, pi))
                    kbs = (2 * pi, 2 * pi + 1)
                    if pi == 0:
                        po_t[h] = pso.tile([D + 1, QT], f32, tag="po",
                                           name="po")
                    po = po_t[h]
                    for kb, n, c0, o in zip(kbs, ns, c0s, os_):
                        nc.tensor.matmul(
                            po[:, c0:QT], v_sb[kb][:, h, :], at[:, o:o + n],
                            start=(kb == 0), stop=(kb == nkb - 1))
                    if pi == npair - 1:
                        # evict numerator into the paired-head tile and the
                        # denominator row into its sg slot; frees the bank
                        j = h // 2
                        if yr2[j] is None:
                            yr2[j] = yrpool.tile([P, QT], f32, tag="yr",
                                                 name="yr")
                        r0 = D * (h % 2)
                        nc.vector.tensor_copy(yr2[j][r0:r0 + D, :], po[0:D, :])
                        nc.vector.tensor_copy(
                            sgs[h // 4][32 * (h % 4):32 * (h % 4) + 1, :],
                            po[D:D + 1, :])
                        po_t[h] = None

                def recip(g):
                    # ~5x faster than the iterative DVE reciprocal; sums are
                    # positive normals so the approx edge cases can't occur
                    nc.vector.reciprocal_approx_fast(rgf[g][:], sgs[g][:])
                    nc.vector.tensor_copy(rgs[g][:], rgf[g][:])

                last = NQT - 1
                zpart = None
                ztile_l = None
                if tt == last:
                    zpart = ztpool.tile([P, 8, QT], f32, tag="zp", name="zp")
                    ztile_l = ztpool.tile([P, 8, QT], f32, tag="zt", name="zt")

                def phase_a(oc):
                    # first half of the final c_proj: runs during the tail
                    # reciprocal so the PE never idles at kernel end
                    ps = pab.tile([P, QT], f32, tag="pab", name="pab")
                    for i in range(2):
                        nc.tensor.matmul(
                            ps[:], wp_sb[:, i, oc * P:(oc + 1) * P],
                            y_sb[i][:, tt * QT:(tt + 1) * QT],
                            start=(i == 0), stop=(i == 1))
                    nc.scalar.activation(zpart[:, oc, :], ps[:], Ident,
                                         bias=bpj_sb[:, oc:oc + 1])

                for s, t in enumerate(tasks):
                    emit_s(*t)
                    if s >= 1:
                        emit_a(*tasks[s - 1])
                        if tt == last and tasks[s - 1] == (3, npair - 1):
                            # final tile: normalize heads 0-3 mid-stream so
                            # only half the chain lands on the kernel tail
                            recip(0)
                            btp_norm(0, tt, yr2)
                            btp_norm(1, tt, yr2)
                    if s >= 2 and fillers:
                        fillers.pop(0)()
                emit_a(*tasks[-1])
                for go in fillers:
                    go()
                if tt == last:
                    # phase A runs on the PE/ACT while the DVE reciprocal
                    # chain for heads 4-7 completes
                    recip(1)
                    for oc in range(8):
                        phase_a(oc)
                    btp_norm(2, tt, yr2)
                    btp_norm(3, tt, yr2)
                    for oc in range(8):
                        ps = pab.tile([P, QT], f32, tag="pab", name="pab")
                        for i in range(2, 4):
                            nc.tensor.matmul(
                                ps[:], wp_sb[:, i, oc * P:(oc + 1) * P],
                                y_sb[i][:, tt * QT:(tt + 1) * QT],
                                start=(i == 2), stop=(i == 3))
                        nc.vector.tensor_add(ztile_l[:, oc, :], ps[:],
                                             zpart[:, oc, :])
                        if oc == 7:
                            nc.sync.dma_start(
                                zT[:, :, tt * QT:(tt + 1) * QT], ztile_l[:])
                else:
                    recip(0)
                    recip(1)
                    prev = (tt, yr2)
    nc.compile()
    return nc


def get_nc():
    if "nc" not in _nc_cache:
        _nc_cache["nc"] = _build_nc()
    return _nc_cache["nc"]


def _mm_np_dtype():
    if MM_DT == "bfloat16":
        import ml_dtypes
        return np.dtype(ml_dtypes.bfloat16)
    return np.dtype(np.float32)


def _blk(a, nb):
    """[nb*128, F] -> [128, nb, F] with out[p, i, f] = a[i*128+p, f]."""
    return np.ascontiguousarray(
        a.reshape(nb, P, -1).transpose(1, 0, 2))


def make_in_maps(x, Wqkv, bqkv, Wproj, bproj):
    x = np.asarray(x, np.float32)
    Wqkv = np.asarray(Wqkv, np.float32)
    bqkv = np.asarray(bqkv, np.float32)
    Wproj = np.asarray(Wproj, np.float32)
    bproj = np.asarray(bproj, np.float32)
    Wq, Wk, Wv = Wqkv[0:C], Wqkv[C:2 * C], Wqkv[2 * C:3 * C]
    bq, bk, bv = bqkv[0:C], bqkv[C:2 * C], bqkv[2 * C:3 * C]
    mdt = _mm_np_dtype()
    mask = (np.arange(QT)[None, :] >= np.arange(P)[:, None]).astype(mdt)
    in_maps = []
    for b in range(B):
        xTb = _blk(x[b].T.astype(mdt), IC)
        for s in range(2):
            cols = slice(s * LC, (s + 1) * LC)
            wqkT = _blk(np.concatenate(
                [Wq[cols], Wk[cols]], 0).T.astype(mdt), IC)
            bqk_ = np.concatenate([bq[cols], bk[cols]])
            wvT_ = _blk(Wv[cols].T.astype(mdt), IC)
            wpT_ = _blk(Wproj[:, cols].T.astype(mdt), 4)
            bp_eff = bv[cols] @ Wproj[:, cols].T
            if s == 0:
                bp_eff = bp_eff + bproj
            in_maps.append({
                "xT": xTb,
                "wqkT": wqkT,
                "bqk": np.ascontiguousarray(bqk_.reshape(8, P).T),
                "wvT": wvT_,
                "wpT": wpT_,
                "bpj": np.ascontiguousarray(
                    bp_eff.astype(np.float32).reshape(8, P).T),
                "mask": mask,
            })
    return in_maps


def gather_out(results):
    out = np.empty((B, T, C), np.float32)
    for b in range(B):
        zt = results[2 * b]["zT"] + results[2 * b + 1]["zT"]
        # zt[p, oc, t] -> z[t, oc*128+p]
        out[b] = zt.transpose(1, 0, 2).reshape(C, T).T
    return out


def kernel(x, Wqkv, bqkv, Wproj, bproj):
    from concourse.bass_utils import run_bass_kernel_spmd

    in_maps = make_in_maps(x, Wqkv, bqkv, Wproj, bproj)
    try:
        res = run_bass_kernel_spmd(get_nc(), in_maps, core_ids=list(range(8)))
    except Exception:
        # transient device faults have been observed once; retry a single time
        res = run_bass_kernel_spmd(get_nc(), in_maps, core_ids=list(range(8)))
    return gather_out(res.results)


# revision 16
# speedup vs baseline: 1.6983x; 1.0315x over previous
"""Causal self-attention (B=4, T=2048, C=1024, H=16) on 8 trn2 NeuronCores.

Sharding: core = (batch b, head-half s).  Each core computes q/k/v
projections for its 8 heads (weights pre-sliced/transposed on host),
causal flash-style attention with transposed score tiles, and a partial
(row-sharded) c_proj.  Host gather sums the two partials per batch.

v4: bf16 matmul inputs; proj/attention merged per token tile; attention
software-pipelined (S-pair emitted one slot ahead of its att@V consumer
so the PE never waits on exp); normalize+cproj of tile tt deferred past
proj(tt+1) so the slow DVE reciprocal never heads the PE queue; host
packs weights/x/z into [128, blk, free] so each tensor is one big DMA.

Device data layout (bf16 unless noted):
  xT    [128, 8, 2048]  x[b].T blocked     xT[p,i,t] = x[b][t, i*128+p]
  wqkT  [128, 8, 1024]  [Wq_l | Wk_l].T blocked
  bqk   [128, 8] f32    q/k bias, per out-ch block
  wvT   [128, 8, 512]   Wv_local.T blocked
  wpT   [128, 4, 1024]  Wproj[:, local].T blocked
  bpj   [128, 8] f32    bproj + bv@WprojT (folded), half per core
  mask  [128, 512]      causal keep-mask m[p,c] = (c >= p)
  zT    [128, 8, 2048] f32  partial out, z[t, oc*128+p] = zT[p,oc,t]
"""

import os
import sys

sys.path.insert(0, "/opt/trn_rl_repo")

import numpy as np

B, T, C, H = 4, 2048, 1024, 16
D = 64          # head dim
NH = 8          # heads per core
LC = NH * D     # local channels = 512
P = 128
QT = 512        # query tile (also matmul moving free dim)
NQT = T // QT   # 4
NKB = T // P    # 16 key blocks
IC = C // P     # 8 input-channel blocks

# matmul input dtype: bfloat16 = full-rate, float32r fallback (exact-ish)
MM_DT = os.environ.get("BASS_ATTN_MM_DT", "bfloat16")

_nc_cache = {}


def _build_nc():
    from contextlib import ExitStack

    import concourse.bass as bass  # noqa: F401
    import concourse.mybir as mybir
    from concourse import bacc, tile

    f32 = mybir.dt.float32
    mdt = getattr(mybir.dt, MM_DT)
    Exp = mybir.ActivationFunctionType.Exp
    Ident = mybir.ActivationFunctionType.Identity
    Copy = mybir.ActivationFunctionType.Copy

    nc = bacc.Bacc("TRN2", target_bir_lowering=False, debug=False, num_devices=8)
    xT = nc.dram_tensor("xT", [P, IC, T], mdt, kind="ExternalInput").ap()
    wqkT = nc.dram_tensor("wqkT", [P, IC, 2 * LC], mdt, kind="ExternalInput").ap()
    bqk = nc.dram_tensor("bqk", [P, 8], f32, kind="ExternalInput").ap()
    wvT = nc.dram_tensor("wvT", [P, IC, LC], mdt, kind="ExternalInput").ap()
    wpT = nc.dram_tensor("wpT", [P, 4, C], mdt, kind="ExternalInput").ap()
    bpj = nc.dram_tensor("bpj", [P, 8], f32, kind="ExternalInput").ap()
    maskd = nc.dram_tensor("mask", [P, QT], mdt, kind="ExternalInput").ap()
    zT = nc.dram_tensor("zT", [P, 8, T], f32, kind="ExternalOutput").ap()

    with tile.TileContext(nc) as tc:
        with ExitStack() as st:
            st.enter_context(nc.allow_low_precision(
                reason="bf16 throughput; accuracy checked vs reference"))
            persist = st.enter_context(tc.tile_pool(name="persist", bufs=1))
            # qk_sb: out-ch blocks 0-3 = q, 4-7 = k; [out-ch 128, tok 2048]
            qk_sb = [persist.tile([P, T], mdt, tag=f"qk{i}", name=f"qk{i}")
                     for i in range(8)]
            # v_sb[kb]: [tok 128, head 8, d 64 + ones col]
            v_sb = [persist.tile([P, NH, D + 1], mdt, tag=f"v{i}", name=f"v{i}")
                    for i in range(NKB)]
            # y_sb: attention out, [local-ch 128, tok 2048] x 4 blocks
            y_sb = [persist.tile([P, T], mdt, tag=f"y{i}", name=f"y{i}")
                    for i in range(4)]
            bqk_sb = persist.tile([P, 8], f32, tag="bqk", name="bqk")
            bpj_sb = persist.tile([P, 8], f32, tag="bpj", name="bpj")
            maskf = persist.tile([P, QT], mdt, tag="maskf", name="maskf")
            # softmax-sum rows: 4 heads per tile at 32-aligned partitions;
            # memset once so dead rows never produce inf/NaN via reciprocal
            sgs = [persist.tile([P, QT], f32, tag=f"sg{i}", name=f"sg{i}")
                   for i in range(2)]
            rgs = [persist.tile([P, QT], mdt, tag=f"rg{i}", name=f"rg{i}")
                   for i in range(2)]
            rgf = [persist.tile([P, QT], f32, tag=f"rf{i}", name=f"rf{i}")
                   for i in range(2)]
            # E matrices: block row-broadcast for paired-head normalization.
            # btp[p,c] = sum_k E[k,p]*rg[k,c]; E_lo rows {0,32}, E_hi {64,96}
            E_bc = [persist.tile([P, P], mdt, tag=f"E{i}", name=f"E{i}")
                    for i in range(2)]
            # weights, host-packed [128, blk, free]
            wqk_sb = persist.tile([P, IC, 2 * LC], mdt, tag="wqk", name="wqk")
            wv_sb = persist.tile([P, IC, LC], mdt, tag="wv", name="wv")
            wp_sb = persist.tile([P, 4, C], mdt, tag="wp", name="wp")

            xpool = st.enter_context(tc.tile_pool(name="xs", bufs=2))
            apool = st.enter_context(tc.tile_pool(name="att", bufs=3))
            yrpool = st.enter_context(tc.tile_pool(name="yraw", bufs=4))
            ztpool = st.enter_context(tc.tile_pool(name="zev", bufs=2))
            # PSUM: pab 2 + ps 2x2 + po 2 = 8 banks
            pab = st.enter_context(tc.tile_pool(name="pab", bufs=2, space="PSUM"))
            pss = st.enter_context(tc.tile_pool(name="pss", bufs=2, space="PSUM"))
            pso = st.enter_context(tc.tile_pool(name="pso", bufs=2, space="PSUM"))

            for i in range(2):
                nc.vector.memset(sgs[i][:], 1.0)
                nc.vector.memset(E_bc[i][:], 0.0)
            nc.vector.memset(E_bc[0][0:1, 0:D], 1.0)
            nc.vector.memset(E_bc[0][32:33, D:P], 1.0)
            nc.vector.memset(E_bc[1][64:65, 0:D], 1.0)
            nc.vector.memset(E_bc[1][96:97, D:P], 1.0)

            def btp_norm(j, ptt, yr2p):
                # heads 2j,2j+1 live in rg tile j//2 rows 64(j%2)+{0,32}
                btp = pab.tile([P, QT], f32, tag="pab", name="btp")
                nc.tensor.matmul(btp[:], E_bc[j % 2][:], rgs[j // 2][:],
                                 start=True, stop=True)
                nc.vector.tensor_mul(
                    y_sb[j][:, ptt * QT:(ptt + 1) * QT], yr2p[j][:], btp[:])

            def cproj_chunks(ptt):
                # one thunk per output-channel block; interleaved into the
                # attention stream as PE gap-filler so the HAM clock gate
                # never sees an idle window during ACT-bound stretches
                ztile = ztpool.tile([P, 8, QT], f32, tag="zt", name="zt")

                def mk(oc):
                    def go():
                        ps = pab.tile([P, QT], f32, tag="pab", name="pab")
                        for i in range(4):
                            nc.tensor.matmul(
                                ps[:], wp_sb[:, i, oc * P:(oc + 1) * P],
                                y_sb[i][:, ptt * QT:(ptt + 1) * QT],
                                start=(i == 0), stop=(i == 3))
                        if oc >= 6:
                            # ACT is idle at tile boundaries; jumping the DVE
                            # backlog here frees the psum slot for proj(tt+1)
                            nc.scalar.activation(
                                ztile[:, oc, :], ps[:], Ident,
                                bias=bpj_sb[:, oc:oc + 1])
                        else:
                            nc.vector.tensor_scalar_add(
                                ztile[:, oc, :], ps[:], bpj_sb[:, oc:oc + 1])
                        if oc == 7:
                            nc.sync.dma_start(
                                zT[:, :, ptt * QT:(ptt + 1) * QT], ztile[:])
                    return go
                return [mk(oc) for oc in range(8)]

            prev = None
            for tt in range(NQT):
                # ---- projections for token tile tt ----
                xt = xpool.tile([P, IC, QT], mdt, tag="x", name="x")
                if tt == 0:
                    # interleaved 2-block chunks pace the first qk chain
                    for c0 in range(0, IC, 2):
                        nc.sync.dma_start(xt[:, c0:c0 + 2, :],
                                          xT[:, c0:c0 + 2, 0:QT])
                        nc.sync.dma_start(wqk_sb[:, c0:c0 + 2, :],
                                          wqkT[:, c0:c0 + 2, :])
                    nc.sync.dma_start(bqk_sb[:], bqk)
                    nc.sync.dma_start(wv_sb[:], wvT)
                    nc.sync.dma_start(maskf[:], maskd)
                    nc.sync.dma_start(wp_sb[:], wpT)
                    nc.sync.dma_start(bpj_sb[:], bpj)
                else:
                    nc.sync.dma_start(xt[:], xT[:, :, tt * QT:(tt + 1) * QT])
                # qk-proj: psum[out-ch 128, tok 512] accumulated over in-ch
                for oc in range(8):
                    ps = pab.tile([P, QT], f32, tag="pab", name="pab")
                    for i in range(IC):
                        nc.tensor.matmul(
                            ps[:], wqk_sb[:, i, oc * P:(oc + 1) * P],
                            xt[:, i, :], start=(i == 0), stop=(i == IC - 1))
                    nc.scalar.activation(
                        qk_sb[oc][:, tt * QT:(tt + 1) * QT], ps[:], Ident,
                        bias=bqk_sb[:, oc:oc + 1])
                # v-proj: psum[tok 128, out-ch 512] per tok block
                for tb in range(4):
                    kb = tt * 4 + tb
                    ps = pab.tile([P, NH, D], f32, tag="pab", name="pab")
                    for i in range(IC):
                        nc.tensor.matmul(
                            ps[:], xt[:, i, tb * P:(tb + 1) * P],
                            wv_sb[:, i, :], start=(i == 0), stop=(i == IC - 1))
                    nc.scalar.activation(v_sb[kb][:, :, 0:D], ps[:], Copy)
                    nc.vector.memset(v_sb[kb][:, :, D:D + 1], 1.0)

                # deferred normalize + c_proj for the previous token tile:
                # runs while this tile's attention streams, so the PE queue
                # never stalls behind the reciprocal
                fillers = []
                if prev is not None:
                    pvt, yr2p = prev
                    for j in range(4):
                        fillers.append(
                            lambda j=j: btp_norm(j, pvt, yr2p))
                    fillers += cproj_chunks(pvt)

                # ---- attention for query tile tt ----
                qtt = tt
                nkb = (qtt + 1) * 4
                # task list: (head-pair j, kb).  The two heads of a pair have
                # their K-blocks at partition offsets 0/64, so the two K=64
                # score matmuls land in distinct PE row-groups and stream
                # concurrently (row tiling); emitted one slot ahead of att@V
                tasks = [(j, kb) for j in range(4) for kb in range(nkb)]
                po_t = {}
                at_t = {}
                yr2 = [None] * 4

                def emit_s(j, kb):
                    qt_i = j
                    kt_i = 4 + j
                    e = kb * P - qtt * QT
                    c0 = max(e, 0)
                    n = QT - c0
                    width = QT + n
                    ps = pss.tile([P, 2 * QT], f32, tag="ps", name="ps")
                    at = apool.tile([P, 2 * QT], mdt, tag="at", name="at")
                    for hh, o in ((0, 0), (1, QT)):
                        p0 = hh * D
                        nc.tensor.matmul(
                            ps[:, o:o + n],
                            qk_sb[kt_i][p0:p0 + D, kb * P:(kb + 1) * P],
                            qk_sb[qt_i][p0:p0 + D,
                                        qtt * QT + c0:(qtt + 1) * QT],
                            start=True, stop=True)
                    nc.scalar.activation(at[:, 0:width], ps[:, 0:width],
                                         Exp, scale=0.125)
                    if e >= 0:
                        # zero strict upper triangle; never reaches past
                        # the first 128 live columns
                        m = min(n, P)
                        for o in (0, QT):
                            nc.vector.tensor_mul(at[:, o:o + m],
                                                 at[:, o:o + m],
                                                 maskf[:, 0:m])
                    at_t[(j, kb)] = (at, n, c0)

                def emit_a(j, kb):
                    at, n, c0 = at_t.pop((j, kb))
                    if kb == 0:
                        po_t[(j, 0)] = pso.tile([D + 1, QT], f32, tag="po",
                                                name="po")
                        po_t[(j, 1)] = pso.tile([D + 1, QT], f32, tag="po",
                                                name="po")
                    for hh, o in ((0, 0), (1, QT)):
                        po = po_t[(j, hh)]
                        nc.tensor.matmul(
                            po[:, c0:QT], v_sb[kb][:, 2 * j + hh, :],
                            at[:, o:o + n],
                            start=(kb == 0), stop=(kb == nkb - 1))
                    if kb == nkb - 1:
                        # evict numerators into the paired-head tile and the
                        # denominator rows into their sg slots; frees banks
                        yr2[j] = yrpool.tile([P, QT], f32, tag="yr",
                                             name="yr")
                        for hh in (0, 1):
                            h = 2 * j + hh
                            po = po_t.pop((j, hh))
                            nc.vector.tensor_copy(
                                yr2[j][hh * D:hh * D + D, :], po[0:D, :])
                            nc.vector.tensor_copy(
                                sgs[h // 4][32 * (h % 4):32 * (h % 4) + 1, :],
                                po[D:D + 1, :])

                def recip(g):
                    # ~5x faster than the iterative DVE reciprocal; sums are
                    # positive normals so the approx edge cases can't occur
                    nc.vector.reciprocal_approx_fast(rgf[g][:], sgs[g][:])
                    nc.vector.tensor_copy(rgs[g][:], rgf[g][:])

                last = NQT - 1
                zpart = None
                ztile_l = None
                if tt == last:
                    zpart = ztpool.tile([P, 8, QT], f32, tag="zp", name="zp")
                    ztile_l = ztpool.tile([P, 8, QT], f32, tag="zt", name="zt")

                def phase_a(oc):
                    # first half of the final c_proj: runs during the tail
                    # reciprocal so the PE never idles at kernel end
                    ps = pab.tile([P, QT], f32, tag="pab", name="pab")
                    for i in range(2):
                        nc.tensor.matmul(
                            ps[:], wp_sb[:, i, oc * P:(oc + 1) * P],
                            y_sb[i][:, tt * QT:(tt + 1) * QT],
                            start=(i == 0), stop=(i == 1))
                    nc.scalar.activation(zpart[:, oc, :], ps[:], Ident,
                                         bias=bpj_sb[:, oc:oc + 1])

                for s, t in enumerate(tasks):
                    emit_s(*t)
                    if s >= 1:
                        emit_a(*tasks[s - 1])
                        if tt == last and tasks[s - 1] == (1, nkb - 1):
                            # final tile: normalize heads 0-3 mid-stream so
                            # only half the chain lands on the kernel tail
                            recip(0)
                            btp_norm(0, tt, yr2)
                            btp_norm(1, tt, yr2)
                    if s >= 2 and fillers:
                        fillers.pop(0)()
                emit_a(*tasks[-1])
                for go in fillers:
                    go()
                if tt == last:
                    # phase A runs on the PE/ACT while the DVE reciprocal
                    # chain for heads 4-7 completes
                    recip(1)
                    for oc in range(8):
                        phase_a(oc)
                    btp_norm(2, tt, yr2)
                    btp_norm(3, tt, yr2)
                    for oc in range(8):
                        ps = pab.tile([P, QT], f32, tag="pab", name="pab")
                        for i in range(2, 4):
                            nc.tensor.matmul(
                                ps[:], wp_sb[:, i, oc * P:(oc + 1) * P],
                                y_sb[i][:, tt * QT:(tt + 1) * QT],
                                start=(i == 2), stop=(i == 3))
                        nc.vector.tensor_add(ztile_l[:, oc, :], ps[:],
                                             zpart[:, oc, :])
                        if oc == 7:
                            nc.sync.dma_start(
                                zT[:, :, tt * QT:(tt + 1) * QT], ztile_l[:])
                else:
                    recip(0)
                    recip(1)
                    prev = (tt, yr2)
    nc.compile()
    return nc


def get_nc():
    if "nc" not in _nc_cache:
        _nc_cache["nc"] = _build_nc()
    return _nc_cache["nc"]


def _mm_np_dtype():
    if MM_DT == "bfloat16":
        import ml_dtypes
        return np.dtype(ml_dtypes.bfloat16)
    return np.dtype(np.float32)


def _blk(a, nb):
    """[nb*128, F] -> [128, nb, F] with out[p, i, f] = a[i*128+p, f]."""
    return np.ascontiguousarray(
        a.reshape(nb, P, -1).transpose(1, 0, 2))


def make_in_maps(x, Wqkv, bqkv, Wproj, bproj):
    x = np.asarray(x, np.float32)
    Wqkv = np.asarray(Wqkv, np.float32)
    bqkv = np.asarray(bqkv, np.float32)
    Wproj = np.asarray(Wproj, np.float32)
    bproj = np.asarray(bproj, np.float32)
    Wq, Wk, Wv = Wqkv[0:C], Wqkv[C:2 * C], Wqkv[2 * C:3 * C]
    bq, bk, bv = bqkv[0:C], bqkv[C:2 * C], bqkv[2 * C:3 * C]
    mdt = _mm_np_dtype()
    mask = (np.arange(QT)[None, :] >= np.arange(P)[:, None]).astype(mdt)
    in_maps = []
    for b in range(B):
        xTb = _blk(x[b].T.astype(mdt), IC)
        for s in range(2):
            cols = slice(s * LC, (s + 1) * LC)
            wqkT = _blk(np.concatenate(
                [Wq[cols], Wk[cols]], 0).T.astype(mdt), IC)
            bqk_ = np.concatenate([bq[cols], bk[cols]])
            wvT_ = _blk(Wv[cols].T.astype(mdt), IC)
            wpT_ = _blk(Wproj[:, cols].T.astype(mdt), 4)
            bp_eff = bv[cols] @ Wproj[:, cols].T
            if s == 0:
                bp_eff = bp_eff + bproj
            in_maps.append({
                "xT": xTb,
                "wqkT": wqkT,
                "bqk": np.ascontiguousarray(bqk_.reshape(8, P).T),
                "wvT": wvT_,
                "wpT": wpT_,
                "bpj": np.ascontiguousarray(
                    bp_eff.astype(np.float32).reshape(8, P).T),
                "mask": mask,
            })
    return in_maps


def gather_out(results):
    out = np.empty((B, T, C), np.float32)
    for b in range(B):
        zt = results[2 * b]["zT"] + results[2 * b + 1]["zT"]
        # zt[p, oc, t] -> z[t, oc*128+p]
        out[b] = zt.transpose(1, 0, 2).reshape(C, T).T
    return out


def kernel(x, Wqkv, bqkv, Wproj, bproj):
    from concourse.bass_utils import run_bass_kernel_spmd

    in_maps = make_in_maps(x, Wqkv, bqkv, Wproj, bproj)
    try:
        res = run_bass_kernel_spmd(get_nc(), in_maps, core_ids=list(range(8)))
    except Exception:
        # transient device faults have been observed once; retry a single time
        res = run_bass_kernel_spmd(get_nc(), in_maps, core_ids=list(range(8)))
    return gather_out(res.results)
